# revision 12
# baseline (speedup 1.0000x reference)
"""Causal self-attention (B=4, T=2048, D=1024, H=16) on 8 TRN2 NeuronCores.

Sharding: core c handles batch b=c//2 and head-group g=c%2 (8 heads).
Each core computes its heads' attention + a partial output projection
(contraction over its 512 attn channels); the host sums the two partials
per batch and adds b_out.

v3: software-pipelined emission tuned for the TRN2 PE p-state (the
tensor engine only reaches 2.4 GHz after 3us of *continuous* busy; any
gap drops it to 1.2 GHz):
  stage 0    qk-proj(pair 0) + v-proj all 16 t-blocks, interleaved
  stage p    attention(pair p-1) with qk-proj(pair p) chains spliced in
             every ~5 key-blocks (covers the S->mask->exp->PV latency)
  stage 4    attention(pair 3) with out-proj of q-chunk qc-1 at each
             q-chunk boundary
Key moves vs the baseline:
  - rope's half-swap runs on the PE as a signed permutation matmul
    (J [128,128]); DVE rope is 2 STT + 1 full-width add per chunk
  - per-head S score tiles in a 4-deep PSUM ring -> 2-block lookahead
  - softmax normalizer 1/Z is computed off the PE-critical path; the
    pv PSUM ring is freed by cheap copies right at the chunk boundary
  - E and V are bf16 (cast on copy; same matmul rate, half the SBUF),
    output DMA is bf16
Pipeline rel-err ~3e-3 vs the 2e-2 gate.
"""
import sys
import numpy as np

for _p in ("/opt/trn_rl_repo", "/root/.axon_site/_ro/trn_rl_repo"):
    if _p not in sys.path:
        sys.path.append(_p)

import concourse.bass as bass
import concourse.bacc as bacc
import concourse.tile as tile
import concourse.mybir as mybir
from concourse import bass_utils

F32 = mybir.dt.float32
F32R = mybir.dt.float32r
BF16 = mybir.dt.bfloat16
AF = mybir.ActivationFunctionType
ALU = mybir.AluOpType

B, T, D, H, DK = 4, 2048, 1024, 16, 64
NC_ = 8          # cores
HPG = 8          # heads per group
NPAIR = 4        # head pairs per core
KT = 8           # 128-row k-tiles over D
XC = 512         # x/qkv t-chunk width
NXC = T // XC    # 4
QC = 512         # attention q-chunk width
NQC = T // QC    # 4
NKB = T // 128   # 16 key blocks
MASK_VAL = -30000.0

_cache = {}


def _build_nc(trace_scopes=False):
    nc = bacc.Bacc("TRN2", target_bir_lowering=False, debug=False)

    xT_d = nc.dram_tensor("xT", [D, T], F32R, kind="ExternalInput").ap()
    wqk_d = nc.dram_tensor("wqk", [D, 1024], F32R, kind="ExternalInput").ap()
    wva_d = nc.dram_tensor("wva", [D, 520], F32R, kind="ExternalInput").ap()
    bva_d = nc.dram_tensor("bva", [1, 520], F32R, kind="ExternalInput").ap()
    ones_d = nc.dram_tensor("ones1", [1, 128], F32R, kind="ExternalInput").ap()
    wo_d = nc.dram_tensor("wo", [512, 1024], F32R, kind="ExternalInput").ap()
    bqk_d = nc.dram_tensor("bqk", [128, 8], F32, kind="ExternalInput").ap()
    cos_d = nc.dram_tensor("cos4", [128, T], F32, kind="ExternalInput").ap()
    sin_d = nc.dram_tensor("sin4", [128, T], F32, kind="ExternalInput").ap()
    jm_d = nc.dram_tensor("Jmat", [128, 128], F32R, kind="ExternalInput").ap()
    out_d = nc.dram_tensor("out", [T, 1024], BF16, kind="ExternalOutput").ap()

    with tile.TileContext(nc, pool_alloc_mode="queue") as tc:
        _emit(tc, nc, xT_d, wqk_d, wva_d, bva_d, ones_d, wo_d, bqk_d,
              cos_d, sin_d, jm_d, out_d)
    nc.compile()
    return nc


def _emit(tc, nc, xT_d, wqk_d, wva_d, bva_d, ones_d, wo_d, bqk_d,
          cos_d, sin_d, jm_d, out_d):
    from contextlib import ExitStack
    ctx = ExitStack()
    with ctx:
        consts = ctx.enter_context(tc.tile_pool(name="consts", bufs=1))
        vpool = ctx.enter_context(tc.tile_pool(name="vpool", bufs=1))
        xcp = ctx.enter_context(tc.tile_pool(name="xcp", bufs=2))
        wqkp = ctx.enter_context(tc.tile_pool(name="wqkp", bufs=2))
        t1p = ctx.enter_context(tc.tile_pool(name="t1p", bufs=2))
        qkp = ctx.enter_context(tc.tile_pool(name="qkp", bufs=8))
        ep = ctx.enter_context(tc.tile_pool(name="ep", bufs=4))
        zbp = ctx.enter_context(tc.tile_pool(name="zbp", bufs=1))
        atp = ctx.enter_context(tc.tile_pool(name="atp", bufs=16))
        outp = ctx.enter_context(tc.tile_pool(name="outp", bufs=3))
        # PSUM: mm ring 2 (qk mmp / rope t2s / v pvm), s ring 4 (per-head
        # score tiles, boundary out-proj), pv ring 2 -> 2+4+2 = 8 banks
        ps_mm = ctx.enter_context(tc.tile_pool(name="ps_mm", bufs=2, space="PSUM"))
        ps_s = ctx.enter_context(tc.tile_pool(name="ps_s", bufs=4, space="PSUM"))
        ps_pv = ctx.enter_context(tc.tile_pool(name="ps_pv", bufs=2, space="PSUM"))

        wqk_r = wqk_d.rearrange("(k p) m -> p k m", p=128)
        xT_r = xT_d.rearrange("(k p) t -> p k t", p=128)

        # ---------------- constants / loads ----------------
        bqk_t = consts.tile([128, 8], F32, tag="bqk")
        nc.sync.dma_start(out=bqk_t[:], in_=bqk_d)
        ones_t = consts.tile([1, 128], F32R, tag="ones")
        nc.sync.dma_start(out=ones_t[:], in_=ones_d)
        bva_t = consts.tile([1, 520], F32R, tag="bva")
        nc.sync.dma_start(out=bva_t[:], in_=bva_d)
        jm_t = consts.tile([128, 128], F32R, tag="jm")
        nc.sync.dma_start(out=jm_t[:], in_=jm_d)

        wqk_tiles = {}

        def load_wqk(p):
            t = wqkp.tile([128, KT, 256], F32R, tag="wqk", name=f"wqk{p}")
            nc.sync.dma_start(out=t[:], in_=wqk_r[:, :, 256 * p:256 * (p + 1)])
            wqk_tiles[p] = t

        xc_tiles = {}

        def load_xc(p, tq):
            t = xcp.tile([128, KT, XC], F32R, tag="xc", name=f"xc{p}_{tq}")
            nc.sync.dma_start(out=t[:], in_=xT_r[:, :, tq * XC:(tq + 1) * XC])
            xc_tiles[(p, tq)] = t

        load_wqk(0)
        load_xc(0, 0)
        cos_t = consts.tile([128, T], F32, tag="cos")
        nc.sync.dma_start(out=cos_t[:], in_=cos_d)
        sin_t = consts.tile([128, T], F32, tag="sin")
        nc.sync.dma_start(out=sin_t[:], in_=sin_d)
        wva_t = consts.tile([128, KT, 520], F32R, tag="wva")
        nc.sync.dma_start(out=wva_t[:], in_=wva_d.rearrange("(k p) m -> p k m", p=128))
        load_wqk(1)
        wo_t = consts.tile([128, 4, 1024], F32R, tag="wo")
        nc.sync.dma_start(out=wo_t[:], in_=wo_d.rearrange("(k p) m -> p k m", p=128))

        # additive causal masks: tri block [128,128] (valid iff c-r>=0) and
        # the d=3 variant [128,256] = [all-masked | tri]
        mask_t = consts.tile([128, 128], F32, tag="mask")
        nc.gpsimd.memset(mask_t[:], 0.0)
        nc.gpsimd.affine_select(
            out=mask_t[:], in_=mask_t[:], compare_op=ALU.is_ge, fill=MASK_VAL,
            base=0, pattern=[[1, 128]], channel_multiplier=-1)
        mask3_t = consts.tile([128, 256], F32, tag="mask3")
        nc.gpsimd.memset(mask3_t[:, 0:128], MASK_VAL)
        nc.gpsimd.memset(mask3_t[:, 128:256], 0.0)
        nc.gpsimd.affine_select(
            out=mask3_t[:, 128:256], in_=mask3_t[:, 128:256], compare_op=ALU.is_ge,
            fill=MASK_VAL, base=0, pattern=[[1, 128]], channel_multiplier=-1)

        # V_aug for all 16 t-blocks: [128 tok, 16 * (8 heads * 65)], bf16
        V_t = vpool.tile([128, NKB, 520], BF16, tag="V")

        # ---------------- pipeline unit generators ----------------
        qk_state = {}

        def qk_units(p):
            """8 units: qk-proj matmul chain + rope for (chunk, m)."""
            qp_ts = [qkp.tile([128, QC], F32R, tag="qp", name=f"qp{p}_{i}")
                     for i in range(NQC)]
            kp_ts = [qkp.tile([128, QC], F32R, tag="kp", name=f"kp{p}_{i}")
                     for i in range(NQC)]
            qk_state[p] = (qp_ts, kp_ts)
            for tq in range(NXC):
                for mloc in (0, 1):
                    def unit(tq=tq, mloc=mloc, qp_ts=qp_ts, kp_ts=kp_ts, p=p):
                        if mloc == 0 and tq + 1 < NXC:
                            load_xc(p, tq + 1)   # prefetch next chunk
                        c0 = tq * XC
                        dest = qp_ts if mloc == 0 else kp_ts
                        msel = 2 * p + mloc
                        mmp = ps_mm.tile([128, XC], F32, tag="mm")
                        wq = wqk_tiles[p]
                        xc = xc_tiles[(p, tq)]
                        for k in range(KT):
                            nc.tensor.matmul(
                                mmp[:], lhsT=wq[:, k, mloc * 128:(mloc + 1) * 128],
                                rhs=xc[:, k, :],
                                start=(k == 0), stop=(k == KT - 1))
                        bcol = bqk_t[:, msel:msel + 1]
                        # T1 = (psum + b) * cos ; T2 = (psum + b) * sin
                        t1 = t1p.tile([128, XC], F32R, tag="t1")
                        nc.vector.scalar_tensor_tensor(
                            t1[:], mmp[:], bcol, cos_t[:, c0:c0 + XC],
                            op0=ALU.add, op1=ALU.mult)
                        t2 = t1p.tile([128, XC], F32R, tag="t2")
                        nc.vector.scalar_tensor_tensor(
                            t2[:], mmp[:], bcol, sin_t[:, c0:c0 + XC],
                            op0=ALU.add, op1=ALU.mult)
                        # signed half-swap on the PE: t2s = J^T @ t2
                        t2s = ps_mm.tile([128, XC], F32, tag="mm", name="t2s")
                        nc.tensor.matmul(t2s[:], lhsT=jm_t[:], rhs=t2[:],
                                         start=True, stop=True)
                        nc.vector.tensor_add(dest[tq][:, 0:XC], t1[:], t2s[:])
                    yield unit

        def v_units():
            """32 units: v-proj half-chains per t-block (pair-0 chunks)."""
            for tb in range(NKB):
                for half in range(2):
                    def unit(tb=tb, half=half):
                        h0 = half * 260
                        xc = xc_tiles[(0, tb // 4)]
                        tb2 = tb % 4
                        pvm = ps_mm.tile([128, 260], F32, tag="mm", name="pvm")
                        for k in range(KT):
                            nc.tensor.matmul(
                                pvm[:], lhsT=xc[:, k, tb2 * 128:(tb2 + 1) * 128],
                                rhs=wva_t[:, k, h0:h0 + 260],
                                start=(k == 0), stop=False)
                        nc.tensor.matmul(pvm[:], lhsT=ones_t[:],
                                         rhs=bva_t[:, h0:h0 + 260],
                                         start=False, stop=True)
                        nc.scalar.copy(V_t[:, tb, h0:h0 + 260], pvm[:])
                    yield unit

        at_tiles = {}

        def out_proj_qc(qc):
            for qb2 in range(4):
                for oc in range(2):
                    po = ps_s.tile([128, 512], F32, tag="s", name="po")
                    for p4 in range(NPAIR):
                        nc.tensor.matmul(
                            po[:],
                            lhsT=at_tiles[p4][qc][:, qb2 * 128:qb2 * 128 + 128],
                            rhs=wo_t[:, p4, oc * 512:(oc + 1) * 512],
                            start=(p4 == 0), stop=(p4 == NPAIR - 1))
                    ot = outp.tile([128, 512], BF16, tag="ot")
                    nc.scalar.copy(ot[:], po[:])
                    qb = qc * 4 + qb2
                    nc.sync.dma_start(out=out_d[qb * 128:(qb + 1) * 128,
                                                oc * 512:(oc + 1) * 512], in_=ot[:])

        def attn_steps(p):
            """Yields ('first'|'blk'|'fin', qc, fn) steps for pair p."""
            qp_ts, kp_ts = qk_state[p]
            at_qs = [atp.tile([128, QC], F32R, tag="attnT", name=f"at{p}_{i}")
                     for i in range(NQC)]
            at_tiles[p] = at_qs
            for qc in range(NQC):
                nkb = 4 * qc + 4
                pvA = ps_pv.tile([65, QC], F32, tag="pv", name=f"pvA{p}_{qc}")
                pvB = ps_pv.tile([65, QC], F32, tag="pv", name=f"pvB{p}_{qc}")
                s_tiles = {}

                def emit_s(kb, qc=qc):
                    d = kb - 4 * qc
                    v0 = 0 if d < 0 else min(128 * d, QC - 256)
                    kq = kp_ts[kb // 4]
                    kc0 = (kb % 4) * 128
                    qq = qp_ts[qc]
                    tiles = []
                    for hh in range(2):
                        sh = ps_s.tile([128, QC], F32, tag="s", name=f"s{hh}")
                        nc.tensor.matmul(
                            sh[:, v0:], lhsT=kq[64 * hh:64 * hh + 64, kc0:kc0 + 128],
                            rhs=qq[64 * hh:64 * hh + 64, v0:],
                            start=True, stop=True, tile_position=(64 * hh, 0))
                        tiles.append(sh)
                    s_tiles[kb] = (tiles, d, v0)

                def first(qc=qc):
                    emit_s(0)
                    emit_s(1)
                yield ("first", qc, first)

                for kb in range(nkb):
                    def step(kb=kb, qc=qc, nkb=nkb, pvA=pvA, pvB=pvB, p=p):
                        tiles, d, v0 = s_tiles.pop(kb)
                        es = []
                        for hh in range(2):
                            sh = tiles[hh]
                            if d == 3:
                                nc.vector.tensor_add(sh[:, 256:512],
                                                     sh[:, 256:512], mask3_t[:])
                            elif d >= 0:
                                nc.vector.tensor_add(sh[:, v0:v0 + 128],
                                                     sh[:, v0:v0 + 128], mask_t[:])
                            e = ep.tile([128, QC], BF16, tag="e")
                            nc.scalar.activation(e[:, v0:], sh[:, v0:],
                                                 AF.Exp, scale=0.125)
                            es.append(e)
                        if kb + 2 < nkb:
                            emit_s(kb + 2)
                        for hh, pv in ((0, pvA), (1, pvB)):
                            nc.tensor.matmul(
                                pv[0:65, v0:],
                                lhsT=V_t[:, kb, (2 * p + hh) * 65:(2 * p + hh) * 65 + 65],
                                rhs=es[hh][:, v0:],
                                start=(kb == 0), stop=(kb == nkb - 1))
                    yield ("blk", qc, step)

                def finalize(qc=qc, pvA=pvA, pvB=pvB, at_qs=at_qs):
                    for hh, pv in ((0, pvA), (1, pvB)):
                        zrow = zbp.tile([1, QC], F32, tag="zrow",
                                        name=f"zrow{hh}")
                        nc.vector.tensor_copy(zrow[:], pv[64:65, :])
                        rz1 = zbp.tile([1, QC], F32, tag="rz1", name=f"rz1{hh}")
                        nc.vector.reciprocal_approx_fast(rz1[:], zrow[:])
                        rzb = zbp.tile([128, QC], F32, tag="rzb", name=f"rzb{hh}")
                        nc.gpsimd.partition_broadcast(rzb[:], rz1[:])
                        sl = at_qs[qc][64 * hh:64 * hh + 64, :]
                        if hh == 0:
                            nc.vector.tensor_mul(sl, pv[0:64, :], rzb[0:64, :])
                        else:
                            nc.vector.tensor_copy(sl, pv[0:64, :])
                            nc.vector.tensor_mul(sl, sl, rzb[64:128, :])
                yield ("fin", qc, finalize)

        # ---------------- pipeline schedule ----------------
        # stage 0: qk-proj(0) + v-proj interleaved (1 qk per 4 v units)
        qgen = qk_units(0)
        vgen = v_units()
        done = False
        while not done:
            u = next(qgen, None)
            if u is None:
                done = True
            else:
                u()
            for _ in range(4):
                u = next(vgen, None)
                if u:
                    u()
        for u in vgen:
            u()

        # stages 1..4: attention(p) with qk-proj(p+1) chains spliced in
        # every 5 key-blocks; stage 4 runs out-proj at chunk boundaries.
        for p in range(NPAIR):
            if p + 2 < NPAIR:
                load_wqk(p + 2)
            if p + 1 < NPAIR:
                fillers = list(qk_units(p + 1))
                load_xc(p + 1, 0)
            else:
                fillers = []
            nblk = 0
            for kind, qc, fn in attn_steps(p):
                if kind == "blk":
                    nblk += 1
                    if fillers and nblk % 5 == 0:
                        fillers.pop(0)()
                elif kind == "fin" and p == NPAIR - 1 and qc > 0:
                    out_proj_qc(qc - 1)
                fn()
            for f in fillers:
                f()
        out_proj_qc(NQC - 1)


def _prep_inputs(x, W_qkv, b_qkv, W_out, cos, sin):
    """Host-side sharding/permutation. Returns list of 8 per-core in_maps."""
    x = np.ascontiguousarray(np.asarray(x, dtype=np.float32))
    W_qkv = np.asarray(W_qkv, dtype=np.float32)
    b_qkv = np.asarray(b_qkv, dtype=np.float32)
    W_out = np.asarray(W_out, dtype=np.float32)
    cos = np.asarray(cos, dtype=np.float32)
    sin = np.asarray(sin, dtype=np.float32)

    xTs = [np.ascontiguousarray(x[b].T) for b in range(B)]
    # rope tables: rows r = table[:, r % 32]
    cosT = np.ascontiguousarray(cos.T)           # [32, T]
    sinT = np.ascontiguousarray(sin.T)
    cos4 = np.ascontiguousarray(np.tile(cosT, (4, 1)))   # [128, T]
    sin4 = np.ascontiguousarray(np.tile(sinT, (4, 1)))
    ones1 = np.ones((1, 128), np.float32)
    # signed half-swap for rope: t2s[r] = -t2[r+32] (r in lo half of each
    # 64-block), +t2[r-32] (hi half); t2s = J^T @ t2
    Jm = np.zeros((128, 128), np.float32)
    for blk in (0, 64):
        for i in range(32):
            Jm[blk + 32 + i, blk + i] = -1.0
            Jm[blk + i, blk + 32 + i] = 1.0

    groups = []
    for g in range(2):
        heads = [g * HPG + i for i in range(HPG)]
        qk_cols = []
        for p in range(NPAIR):
            A, Bh = heads[2 * p], heads[2 * p + 1]
            for base in (0, DK):                  # q block then k block
                for h in (A, Bh):
                    qk_cols += list(3 * DK * h + base + np.arange(0, DK, 2))
                    qk_cols += list(3 * DK * h + base + np.arange(1, DK, 2))
        qk_cols = np.array(qk_cols)
        wqk = np.ascontiguousarray(W_qkv[:, qk_cols])         # [1024, 1024]
        bqk = np.ascontiguousarray(b_qkv[qk_cols].reshape(8, 128).T)  # [128, 8]
        # v with a normalizer ones col interleaved per head: [1024, 8*65]
        wva = np.zeros((D, 520), np.float32)
        bva = np.zeros((1, 520), np.float32)
        for i, h in enumerate(heads):
            vcols = 3 * DK * h + 2 * DK + np.arange(DK)
            wva[:, i * 65:i * 65 + 64] = W_qkv[:, vcols]
            bva[0, i * 65:i * 65 + 64] = b_qkv[vcols]
            bva[0, i * 65 + 64] = 1.0                 # ones column
        wo = np.ascontiguousarray(W_out[g * 512:(g + 1) * 512, :])
        groups.append(dict(wqk=wqk, bqk=bqk, wva=np.ascontiguousarray(wva),
                           bva=bva, wo=wo))

    in_maps = []
    for c in range(NC_):
        b, g = c // 2, c % 2
        gr = groups[g]
        in_maps.append({
            "xT": xTs[b], "wqk": gr["wqk"], "wva": gr["wva"], "bva": gr["bva"],
            "ones1": ones1, "wo": gr["wo"], "bqk": gr["bqk"],
            "cos4": cos4, "sin4": sin4, "Jmat": Jm,
        })
    return in_maps


def run(x, W_qkv, b_qkv, W_out, b_out, cos, sin, trace=False, trace_cores=None):
    """Build/compile (cached), run on 8 cores, return (out, BassKernelResults)."""
    if "nc" not in _cache:
        _cache["nc"] = _build_nc()
    nc = _cache["nc"]
    in_maps = _prep_inputs(x, W_qkv, b_qkv, W_out, cos, sin)
    kw = {}
    if trace:
        kw = dict(trace=True, trace_cores=trace_cores or [0])
    res = bass_utils.run_bass_kernel_spmd(nc, in_maps, core_ids=list(range(NC_)), **kw)
    b_out = np.asarray(b_out, dtype=np.float32)
    out = np.empty((B, T, D), np.float32)
    for b in range(B):
        out[b] = (res.results[2 * b]["out"].astype(np.float32)
                  + res.results[2 * b + 1]["out"].astype(np.float32)
                  + b_out[None, :])
    return out, res


def kernel(x, W_qkv, b_qkv, W_out, b_out, cos, sin):
    out, _ = run(x, W_qkv, b_qkv, W_out, b_out, cos, sin)
    return out


# revision 13
# speedup vs baseline: 1.2037x; 1.2037x over previous
"""Causal self-attention (B=4, T=2048, D=1024, H=16) on 8 TRN2 NeuronCores.

Sharding: core c handles batch b=c//2 and head-group g=c%2 (8 heads).
Each core computes its heads' attention + a partial output projection
(contraction over its 512 attn channels); the host sums the two partials
per batch and adds b_out.

v3: software-pipelined emission tuned for the TRN2 PE p-state (the
tensor engine only reaches 2.4 GHz after 3us of *continuous* busy; any
gap drops it to 1.2 GHz):
  stage 0    qk-proj(pair 0) + v-proj all 16 t-blocks, interleaved
  stage p    attention(pair p-1) with qk-proj(pair p) chains spliced in
             every ~5 key-blocks (covers the S->mask->exp->PV latency)
  stage 4    attention(pair 3) with out-proj of q-chunk qc-1 at each
             q-chunk boundary
Key moves vs the baseline:
  - rope's half-swap runs on the PE as a signed permutation matmul
    (J [128,128]); DVE rope is 2 STT + 1 full-width add per chunk
  - per-head S score tiles in a 4-deep PSUM ring -> 2-block lookahead
  - softmax normalizer 1/Z is computed off the PE-critical path; the
    pv PSUM ring is freed by cheap copies right at the chunk boundary
  - E and V are bf16 (cast on copy; same matmul rate, half the SBUF),
    output DMA is bf16
Pipeline rel-err ~3e-3 vs the 2e-2 gate.
"""
import sys
import numpy as np

for _p in ("/opt/trn_rl_repo", "/root/.axon_site/_ro/trn_rl_repo"):
    if _p not in sys.path:
        sys.path.append(_p)

import concourse.bass as bass
import concourse.bacc as bacc
import concourse.tile as tile
import concourse.mybir as mybir
from concourse import bass_utils

F32 = mybir.dt.float32
F32R = mybir.dt.float32r
BF16 = mybir.dt.bfloat16
AF = mybir.ActivationFunctionType
ALU = mybir.AluOpType

B, T, D, H, DK = 4, 2048, 1024, 16, 64
NC_ = 8          # cores
HPG = 8          # heads per group
NPAIR = 4        # head pairs per core
KT = 8           # 128-row k-tiles over D
XC = 512         # x/qkv t-chunk width
NXC = T // XC    # 4
QC = 512         # attention q-chunk width
NQC = T // QC    # 4
NKB = T // 128   # 16 key blocks
MASK_VAL = -30000.0

_cache = {}


def _build_nc(trace_scopes=False):
    nc = bacc.Bacc("TRN2", target_bir_lowering=False, debug=False)

    xT_d = nc.dram_tensor("xT", [D, T], BF16, kind="ExternalInput").ap()
    wqk_d = nc.dram_tensor("wqk", [D, 1024], BF16, kind="ExternalInput").ap()
    wva_d = nc.dram_tensor("wva", [D, 520], BF16, kind="ExternalInput").ap()
    bva_d = nc.dram_tensor("bva", [1, 520], BF16, kind="ExternalInput").ap()
    ones_d = nc.dram_tensor("ones1", [1, 128], BF16, kind="ExternalInput").ap()
    wo_d = nc.dram_tensor("wo", [512, 1024], BF16, kind="ExternalInput").ap()
    bqk_d = nc.dram_tensor("bqk", [128, 8], F32, kind="ExternalInput").ap()
    cos_d = nc.dram_tensor("cos4", [128, T], F32, kind="ExternalInput").ap()
    sin_d = nc.dram_tensor("sin4", [128, T], F32, kind="ExternalInput").ap()
    jm_d = nc.dram_tensor("Jmat", [128, 128], BF16, kind="ExternalInput").ap()
    out_d = nc.dram_tensor("out", [T, 1024], BF16, kind="ExternalOutput").ap()

    with tile.TileContext(nc, pool_alloc_mode="queue") as tc:
        _emit(tc, nc, xT_d, wqk_d, wva_d, bva_d, ones_d, wo_d, bqk_d,
              cos_d, sin_d, jm_d, out_d)
    nc.compile()
    return nc


def _emit(tc, nc, xT_d, wqk_d, wva_d, bva_d, ones_d, wo_d, bqk_d,
          cos_d, sin_d, jm_d, out_d):
    from contextlib import ExitStack
    ctx = ExitStack()
    with ctx:
        consts = ctx.enter_context(tc.tile_pool(name="consts", bufs=1))
        vpool = ctx.enter_context(tc.tile_pool(name="vpool", bufs=1))
        xcp = ctx.enter_context(tc.tile_pool(name="xcp", bufs=2))
        wqkp = ctx.enter_context(tc.tile_pool(name="wqkp", bufs=2))
        t1p = ctx.enter_context(tc.tile_pool(name="t1p", bufs=2))
        qkp = ctx.enter_context(tc.tile_pool(name="qkp", bufs=8))
        ep = ctx.enter_context(tc.tile_pool(name="ep", bufs=4))
        zbp = ctx.enter_context(tc.tile_pool(name="zbp", bufs=1))
        atp = ctx.enter_context(tc.tile_pool(name="atp", bufs=16))
        outp = ctx.enter_context(tc.tile_pool(name="outp", bufs=3))
        # PSUM: mm ring 2 (qk mmp / rope t2s / v pvm), s ring 4 (per-head
        # score tiles, boundary out-proj), pv ring 2 -> 2+4+2 = 8 banks
        ps_mm = ctx.enter_context(tc.tile_pool(name="ps_mm", bufs=2, space="PSUM"))
        ps_s = ctx.enter_context(tc.tile_pool(name="ps_s", bufs=4, space="PSUM"))
        ps_pv = ctx.enter_context(tc.tile_pool(name="ps_pv", bufs=2, space="PSUM"))

        wqk_r = wqk_d.rearrange("(k p) m -> p k m", p=128)
        xT_r = xT_d.rearrange("(k p) t -> p k t", p=128)

        # ---------------- constants / loads ----------------
        bqk_t = consts.tile([128, 8], F32, tag="bqk")
        nc.sync.dma_start(out=bqk_t[:], in_=bqk_d)
        ones_t = consts.tile([1, 128], BF16, tag="ones")
        nc.sync.dma_start(out=ones_t[:], in_=ones_d)
        bva_t = consts.tile([1, 520], BF16, tag="bva")
        nc.sync.dma_start(out=bva_t[:], in_=bva_d)
        jm_t = consts.tile([128, 128], BF16, tag="jm")
        nc.sync.dma_start(out=jm_t[:], in_=jm_d)

        wqk_tiles = {}

        def load_wqk(p):
            t = wqkp.tile([128, KT, 256], BF16, tag="wqk", name=f"wqk{p}")
            nc.sync.dma_start(out=t[:], in_=wqk_r[:, :, 256 * p:256 * (p + 1)])
            wqk_tiles[p] = t

        xc_tiles = {}

        def load_xc(p, tq):
            t = xcp.tile([128, KT, XC], BF16, tag="xc", name=f"xc{p}_{tq}")
            nc.sync.dma_start(out=t[:], in_=xT_r[:, :, tq * XC:(tq + 1) * XC])
            xc_tiles[(p, tq)] = t

        load_wqk(0)
        load_xc(0, 0)
        cos_t = consts.tile([128, T], F32, tag="cos")
        nc.sync.dma_start(out=cos_t[:], in_=cos_d)
        sin_t = consts.tile([128, T], F32, tag="sin")
        nc.sync.dma_start(out=sin_t[:], in_=sin_d)
        wva_t = consts.tile([128, KT, 520], BF16, tag="wva")
        nc.sync.dma_start(out=wva_t[:], in_=wva_d.rearrange("(k p) m -> p k m", p=128))
        load_wqk(1)
        wo_t = consts.tile([128, 4, 1024], BF16, tag="wo")
        nc.sync.dma_start(out=wo_t[:], in_=wo_d.rearrange("(k p) m -> p k m", p=128))

        # additive causal masks: tri block [128,128] (valid iff c-r>=0) and
        # the d=3 variant [128,256] = [all-masked | tri]
        mask_t = consts.tile([128, 128], F32, tag="mask")
        nc.gpsimd.memset(mask_t[:], 0.0)
        nc.gpsimd.affine_select(
            out=mask_t[:], in_=mask_t[:], compare_op=ALU.is_ge, fill=MASK_VAL,
            base=0, pattern=[[1, 128]], channel_multiplier=-1)
        mask3_t = consts.tile([128, 256], F32, tag="mask3")
        nc.gpsimd.memset(mask3_t[:, 0:128], MASK_VAL)
        nc.gpsimd.memset(mask3_t[:, 128:256], 0.0)
        nc.gpsimd.affine_select(
            out=mask3_t[:, 128:256], in_=mask3_t[:, 128:256], compare_op=ALU.is_ge,
            fill=MASK_VAL, base=0, pattern=[[1, 128]], channel_multiplier=-1)

        # V_aug for all 16 t-blocks: [128 tok, 16 * (8 heads * 65)], bf16
        V_t = vpool.tile([128, NKB, 520], BF16, tag="V")

        # ---------------- pipeline unit generators ----------------
        qk_state = {}

        def qk_units(p):
            """8 units: qk-proj matmul chain + rope for (chunk, m)."""
            qp_ts = [qkp.tile([128, QC], BF16, tag="qp", name=f"qp{p}_{i}")
                     for i in range(NQC)]
            kp_ts = [qkp.tile([128, QC], BF16, tag="kp", name=f"kp{p}_{i}")
                     for i in range(NQC)]
            qk_state[p] = (qp_ts, kp_ts)
            for tq in range(NXC):
                for mloc in (0, 1):
                    def unit(tq=tq, mloc=mloc, qp_ts=qp_ts, kp_ts=kp_ts, p=p):
                        if mloc == 0 and tq + 1 < NXC:
                            load_xc(p, tq + 1)   # prefetch next chunk
                        c0 = tq * XC
                        dest = qp_ts if mloc == 0 else kp_ts
                        msel = 2 * p + mloc
                        mmp = ps_mm.tile([128, XC], F32, tag="mm")
                        wq = wqk_tiles[p]
                        xc = xc_tiles[(p, tq)]
                        for k in range(KT):
                            nc.tensor.matmul(
                                mmp[:], lhsT=wq[:, k, mloc * 128:(mloc + 1) * 128],
                                rhs=xc[:, k, :],
                                start=(k == 0), stop=(k == KT - 1))
                        bcol = bqk_t[:, msel:msel + 1]
                        # T1 = (psum + b) * cos ; T2 = (psum + b) * sin
                        t1 = t1p.tile([128, XC], BF16, tag="t1")
                        nc.vector.scalar_tensor_tensor(
                            t1[:], mmp[:], bcol, cos_t[:, c0:c0 + XC],
                            op0=ALU.add, op1=ALU.mult)
                        t2 = t1p.tile([128, XC], BF16, tag="t2")
                        nc.vector.scalar_tensor_tensor(
                            t2[:], mmp[:], bcol, sin_t[:, c0:c0 + XC],
                            op0=ALU.add, op1=ALU.mult)
                        # signed half-swap on the PE: t2s = J^T @ t2
                        t2s = ps_mm.tile([128, XC], F32, tag="mm", name="t2s")
                        nc.tensor.matmul(t2s[:], lhsT=jm_t[:], rhs=t2[:],
                                         start=True, stop=True)
                        nc.vector.tensor_add(dest[tq][:, 0:XC], t1[:], t2s[:])
                    yield unit

        def v_units():
            """32 units: v-proj half-chains per t-block (pair-0 chunks)."""
            for tb in range(NKB):
                for half in range(2):
                    def unit(tb=tb, half=half):
                        h0 = half * 260
                        xc = xc_tiles[(0, tb // 4)]
                        tb2 = tb % 4
                        pvm = ps_mm.tile([128, 260], F32, tag="mm", name="pvm")
                        for k in range(KT):
                            nc.tensor.matmul(
                                pvm[:], lhsT=xc[:, k, tb2 * 128:(tb2 + 1) * 128],
                                rhs=wva_t[:, k, h0:h0 + 260],
                                start=(k == 0), stop=False)
                        nc.tensor.matmul(pvm[:], lhsT=ones_t[:],
                                         rhs=bva_t[:, h0:h0 + 260],
                                         start=False, stop=True)
                        nc.scalar.copy(V_t[:, tb, h0:h0 + 260], pvm[:])
                    yield unit

        at_tiles = {}

        def out_proj_qc(qc):
            for qb2 in range(4):
                for oc in range(2):
                    po = ps_s.tile([128, 512], F32, tag="s", name="po")
                    for p4 in range(NPAIR):
                        nc.tensor.matmul(
                            po[:],
                            lhsT=at_tiles[p4][qc][:, qb2 * 128:qb2 * 128 + 128],
                            rhs=wo_t[:, p4, oc * 512:(oc + 1) * 512],
                            start=(p4 == 0), stop=(p4 == NPAIR - 1))
                    ot = outp.tile([128, 512], BF16, tag="ot")
                    nc.scalar.copy(ot[:], po[:])
                    qb = qc * 4 + qb2
                    nc.sync.dma_start(out=out_d[qb * 128:(qb + 1) * 128,
                                                oc * 512:(oc + 1) * 512], in_=ot[:])

        def attn_steps(p):
            """Yields ('first'|'blk'|'fin', qc, fn) steps for pair p."""
            qp_ts, kp_ts = qk_state[p]
            at_qs = [atp.tile([128, QC], BF16, tag="attnT", name=f"at{p}_{i}")
                     for i in range(NQC)]
            at_tiles[p] = at_qs
            for qc in range(NQC):
                nkb = 4 * qc + 4
                pvA = ps_pv.tile([65, QC], F32, tag="pv", name=f"pvA{p}_{qc}")
                pvB = ps_pv.tile([65, QC], F32, tag="pv", name=f"pvB{p}_{qc}")
                s_tiles = {}

                def emit_s(kb, qc=qc):
                    d = kb - 4 * qc
                    v0 = 0 if d < 0 else min(128 * d, QC - 256)
                    kq = kp_ts[kb // 4]
                    kc0 = (kb % 4) * 128
                    qq = qp_ts[qc]
                    tiles = []
                    for hh in range(2):
                        sh = ps_s.tile([128, QC], F32, tag="s", name=f"s{hh}")
                        nc.tensor.matmul(
                            sh[:, v0:], lhsT=kq[64 * hh:64 * hh + 64, kc0:kc0 + 128],
                            rhs=qq[64 * hh:64 * hh + 64, v0:],
                            start=True, stop=True, tile_position=(64 * hh, 0))
                        tiles.append(sh)
                    s_tiles[kb] = (tiles, d, v0)

                def first(qc=qc):
                    emit_s(0)
                    emit_s(1)
                yield ("first", qc, first)

                for kb in range(nkb):
                    def step(kb=kb, qc=qc, nkb=nkb, pvA=pvA, pvB=pvB, p=p):
                        tiles, d, v0 = s_tiles.pop(kb)
                        es = []
                        for hh in range(2):
                            sh = tiles[hh]
                            if d == 3:
                                nc.vector.tensor_add(sh[:, 256:512],
                                                     sh[:, 256:512], mask3_t[:])
                            elif d >= 0:
                                nc.vector.tensor_add(sh[:, v0:v0 + 128],
                                                     sh[:, v0:v0 + 128], mask_t[:])
                            e = ep.tile([128, QC], BF16, tag="e")
                            nc.scalar.activation(e[:, v0:], sh[:, v0:],
                                                 AF.Exp, scale=0.125)
                            es.append(e)
                        if kb + 2 < nkb:
                            emit_s(kb + 2)
                        for hh, pv in ((0, pvA), (1, pvB)):
                            nc.tensor.matmul(
                                pv[0:65, v0:],
                                lhsT=V_t[:, kb, (2 * p + hh) * 65:(2 * p + hh) * 65 + 65],
                                rhs=es[hh][:, v0:],
                                start=(kb == 0), stop=(kb == nkb - 1))
                    yield ("blk", qc, step)

                def finalize(qc=qc, pvA=pvA, pvB=pvB, at_qs=at_qs):
                    for hh, pv in ((0, pvA), (1, pvB)):
                        zrow = zbp.tile([1, QC], F32, tag="zrow",
                                        name=f"zrow{hh}")
                        nc.vector.tensor_copy(zrow[:], pv[64:65, :])
                        rz1 = zbp.tile([1, QC], F32, tag="rz1", name=f"rz1{hh}")
                        nc.vector.reciprocal_approx_fast(rz1[:], zrow[:])
                        rzb = zbp.tile([128, QC], F32, tag="rzb", name=f"rzb{hh}")
                        nc.gpsimd.partition_broadcast(rzb[:], rz1[:])
                        sl = at_qs[qc][64 * hh:64 * hh + 64, :]
                        if hh == 0:
                            nc.vector.tensor_mul(sl, pv[0:64, :], rzb[0:64, :])
                        else:
                            nc.vector.tensor_copy(sl, pv[0:64, :])
                            nc.vector.tensor_mul(sl, sl, rzb[64:128, :])
                yield ("fin", qc, finalize)

        # ---------------- pipeline schedule ----------------
        # stage 0: qk-proj(0) + v-proj interleaved (1 qk per 4 v units)
        qgen = qk_units(0)
        vgen = v_units()
        done = False
        while not done:
            u = next(qgen, None)
            if u is None:
                done = True
            else:
                u()
            for _ in range(4):
                u = next(vgen, None)
                if u:
                    u()
        for u in vgen:
            u()

        # stages 1..4: attention(p) with qk-proj(p+1) chains spliced in
        # every 5 key-blocks; stage 4 runs out-proj at chunk boundaries.
        for p in range(NPAIR):
            if p + 2 < NPAIR:
                load_wqk(p + 2)
            if p + 1 < NPAIR:
                fillers = list(qk_units(p + 1))
                load_xc(p + 1, 0)
            else:
                fillers = []
            nblk = 0
            for kind, qc, fn in attn_steps(p):
                if kind == "blk":
                    nblk += 1
                    if fillers and nblk % 5 == 0:
                        fillers.pop(0)()
                elif kind == "fin" and p == NPAIR - 1 and qc > 0:
                    out_proj_qc(qc - 1)
                fn()
            for f in fillers:
                f()
        out_proj_qc(NQC - 1)


def _prep_inputs(x, W_qkv, b_qkv, W_out, cos, sin):
    """Host-side sharding/permutation. Returns list of 8 per-core in_maps."""
    x = np.ascontiguousarray(np.asarray(x, dtype=np.float32))
    W_qkv = np.asarray(W_qkv, dtype=np.float32)
    b_qkv = np.asarray(b_qkv, dtype=np.float32)
    W_out = np.asarray(W_out, dtype=np.float32)
    cos = np.asarray(cos, dtype=np.float32)
    sin = np.asarray(sin, dtype=np.float32)

    import ml_dtypes
    BF = ml_dtypes.bfloat16
    xTs = [np.ascontiguousarray(x[b].T).astype(BF) for b in range(B)]
    # rope tables: rows r = table[:, r % 32]
    cosT = np.ascontiguousarray(cos.T)           # [32, T]
    sinT = np.ascontiguousarray(sin.T)
    cos4 = np.ascontiguousarray(np.tile(cosT, (4, 1)))   # [128, T]
    sin4 = np.ascontiguousarray(np.tile(sinT, (4, 1)))
    ones1 = np.ones((1, 128), BF)
    # signed half-swap for rope: t2s[r] = -t2[r+32] (r in lo half of each
    # 64-block), +t2[r-32] (hi half); t2s = J^T @ t2
    Jm = np.zeros((128, 128), np.float32)  # cast to BF below
    for blk in (0, 64):
        for i in range(32):
            Jm[blk + 32 + i, blk + i] = -1.0
            Jm[blk + i, blk + 32 + i] = 1.0

    groups = []
    for g in range(2):
        heads = [g * HPG + i for i in range(HPG)]
        qk_cols = []
        for p in range(NPAIR):
            A, Bh = heads[2 * p], heads[2 * p + 1]
            for base in (0, DK):                  # q block then k block
                for h in (A, Bh):
                    qk_cols += list(3 * DK * h + base + np.arange(0, DK, 2))
                    qk_cols += list(3 * DK * h + base + np.arange(1, DK, 2))
        qk_cols = np.array(qk_cols)
        wqk = np.ascontiguousarray(W_qkv[:, qk_cols]).astype(BF)  # [1024, 1024]
        bqk = np.ascontiguousarray(b_qkv[qk_cols].reshape(8, 128).T)  # [128, 8]
        # v with a normalizer ones col interleaved per head: [1024, 8*65]
        wva = np.zeros((D, 520), np.float32)
        bva = np.zeros((1, 520), np.float32)
        for i, h in enumerate(heads):
            vcols = 3 * DK * h + 2 * DK + np.arange(DK)
            wva[:, i * 65:i * 65 + 64] = W_qkv[:, vcols]
            bva[0, i * 65:i * 65 + 64] = b_qkv[vcols]
            bva[0, i * 65 + 64] = 1.0                 # ones column
        wo = np.ascontiguousarray(W_out[g * 512:(g + 1) * 512, :]).astype(BF)
        groups.append(dict(wqk=wqk, bqk=bqk, wva=wva.astype(BF),
                           bva=bva.astype(BF), wo=wo))

    in_maps = []
    for c in range(NC_):
        b, g = c // 2, c % 2
        gr = groups[g]
        in_maps.append({
            "xT": xTs[b], "wqk": gr["wqk"], "wva": gr["wva"], "bva": gr["bva"],
            "ones1": ones1, "wo": gr["wo"], "bqk": gr["bqk"],
            "cos4": cos4, "sin4": sin4, "Jmat": Jm.astype(BF),
        })
    return in_maps


def run(x, W_qkv, b_qkv, W_out, b_out, cos, sin, trace=False, trace_cores=None):
    """Build/compile (cached), run on 8 cores, return (out, BassKernelResults)."""
    if "nc" not in _cache:
        _cache["nc"] = _build_nc()
    nc = _cache["nc"]
    in_maps = _prep_inputs(x, W_qkv, b_qkv, W_out, cos, sin)
    kw = {}
    if trace:
        kw = dict(trace=True, trace_cores=trace_cores or [0])
    res = bass_utils.run_bass_kernel_spmd(nc, in_maps, core_ids=list(range(NC_)), **kw)
    b_out = np.asarray(b_out, dtype=np.float32)
    out = np.empty((B, T, D), np.float32)
    for b in range(B):
        out[b] = (res.results[2 * b]["out"].astype(np.float32)
                  + res.results[2 * b + 1]["out"].astype(np.float32)
                  + b_out[None, :])
    return out, res


def kernel(x, W_qkv, b_qkv, W_out, b_out, cos, sin):
    out, _ = run(x, W_qkv, b_qkv, W_out, b_out, cos, sin)
    return out


# revision 15
# speedup vs baseline: 1.2311x; 1.0228x over previous
"""Causal self-attention (B=4, T=2048, D=1024, H=16) on 8 TRN2 NeuronCores.

Sharding: core c handles batch b=c//2 and head-group g=c%2 (8 heads).
Each core computes its heads' attention + a partial output projection
(contraction over its 512 attn channels); the host sums the two partials
per batch and adds b_out.

v3: software-pipelined emission tuned for the TRN2 PE p-state (the
tensor engine only reaches 2.4 GHz after 3us of *continuous* busy; any
gap drops it to 1.2 GHz):
  stage 0    qk-proj(pair 0) + v-proj all 16 t-blocks, interleaved
  stage p    attention(pair p-1) with qk-proj(pair p) chains spliced in
             every ~5 key-blocks (covers the S->mask->exp->PV latency)
  stage 4    attention(pair 3) with out-proj of q-chunk qc-1 at each
             q-chunk boundary
Key moves vs the baseline:
  - rope's half-swap runs on the PE as a signed permutation matmul
    (J [128,128]); DVE rope is 2 STT + 1 full-width add per chunk
  - per-head S score tiles in a 4-deep PSUM ring -> 2-block lookahead
  - softmax normalizer 1/Z is computed off the PE-critical path; the
    pv PSUM ring is freed by cheap copies right at the chunk boundary
  - E and V are bf16 (cast on copy; same matmul rate, half the SBUF),
    output DMA is bf16
Pipeline rel-err ~3e-3 vs the 2e-2 gate.
"""
import sys
import numpy as np

for _p in ("/opt/trn_rl_repo", "/root/.axon_site/_ro/trn_rl_repo"):
    if _p not in sys.path:
        sys.path.append(_p)

import concourse.bass as bass
import concourse.bacc as bacc
import concourse.tile as tile
import concourse.mybir as mybir
from concourse import bass_utils

F32 = mybir.dt.float32
F32R = mybir.dt.float32r
BF16 = mybir.dt.bfloat16
AF = mybir.ActivationFunctionType
ALU = mybir.AluOpType

B, T, D, H, DK = 4, 2048, 1024, 16, 64
NC_ = 8          # cores
HPG = 8          # heads per group
NPAIR = 4        # head pairs per core
KT = 8           # 128-row k-tiles over D
XC = 512         # x/qkv t-chunk width
NXC = T // XC    # 4
QC = 512         # attention q-chunk width
NQC = T // QC    # 4
NKB = T // 128   # 16 key blocks
MASK_VAL = -30000.0

_cache = {}


def _build_nc(trace_scopes=False):
    nc = bacc.Bacc("TRN2", target_bir_lowering=False, debug=False)

    xT_d = nc.dram_tensor("xT", [D, T], BF16, kind="ExternalInput").ap()
    wqk_d = nc.dram_tensor("wqk", [D, 1024], BF16, kind="ExternalInput").ap()
    wva_d = nc.dram_tensor("wva", [D, 520], BF16, kind="ExternalInput").ap()
    bva_d = nc.dram_tensor("bva", [1, 520], BF16, kind="ExternalInput").ap()
    ones_d = nc.dram_tensor("ones1", [1, 128], BF16, kind="ExternalInput").ap()
    wo_d = nc.dram_tensor("wo", [512, 1024], BF16, kind="ExternalInput").ap()
    bqk_d = nc.dram_tensor("bqk", [128, 8], F32, kind="ExternalInput").ap()
    cos_d = nc.dram_tensor("cos4", [128, T], F32, kind="ExternalInput").ap()
    sin_d = nc.dram_tensor("sin4", [128, T], F32, kind="ExternalInput").ap()
    jm_d = nc.dram_tensor("Jmat", [128, 128], BF16, kind="ExternalInput").ap()
    out_d = nc.dram_tensor("out", [T, 1024], BF16, kind="ExternalOutput").ap()

    with tile.TileContext(nc, pool_alloc_mode="queue") as tc:
        _emit(tc, nc, xT_d, wqk_d, wva_d, bva_d, ones_d, wo_d, bqk_d,
              cos_d, sin_d, jm_d, out_d)
    nc.compile()
    return nc


def _emit(tc, nc, xT_d, wqk_d, wva_d, bva_d, ones_d, wo_d, bqk_d,
          cos_d, sin_d, jm_d, out_d):
    from contextlib import ExitStack
    ctx = ExitStack()
    with ctx:
        consts = ctx.enter_context(tc.tile_pool(name="consts", bufs=1))
        vpool = ctx.enter_context(tc.tile_pool(name="vpool", bufs=1))
        xcp = ctx.enter_context(tc.tile_pool(name="xcp", bufs=2))
        wqkp = ctx.enter_context(tc.tile_pool(name="wqkp", bufs=2))
        t1p = ctx.enter_context(tc.tile_pool(name="t1p", bufs=2))
        qkp = ctx.enter_context(tc.tile_pool(name="qkp", bufs=8))
        ep = ctx.enter_context(tc.tile_pool(name="ep", bufs=4))
        zbp = ctx.enter_context(tc.tile_pool(name="zbp", bufs=1))
        atp = ctx.enter_context(tc.tile_pool(name="atp", bufs=16))
        outp = ctx.enter_context(tc.tile_pool(name="outp", bufs=3))
        # PSUM: mm ring 2 (qk mmp / rope t2s / v pvm), s ring 4 (per-head
        # score tiles, boundary out-proj), pv ring 2 -> 2+4+2 = 8 banks
        ps_mm = ctx.enter_context(tc.tile_pool(name="ps_mm", bufs=2, space="PSUM"))
        ps_s = ctx.enter_context(tc.tile_pool(name="ps_s", bufs=4, space="PSUM"))
        ps_pv = ctx.enter_context(tc.tile_pool(name="ps_pv", bufs=2, space="PSUM"))

        wqk_r = wqk_d.rearrange("(k p) m -> p k m", p=128)
        xT_r = xT_d.rearrange("(k p) t -> p k t", p=128)

        # ---------------- constants / loads ----------------
        bqk_t = consts.tile([128, 8], F32, tag="bqk")
        nc.sync.dma_start(out=bqk_t[:], in_=bqk_d)
        ones_t = consts.tile([1, 128], BF16, tag="ones")
        nc.sync.dma_start(out=ones_t[:], in_=ones_d)
        bva_t = consts.tile([1, 520], BF16, tag="bva")
        nc.sync.dma_start(out=bva_t[:], in_=bva_d)
        jm_t = consts.tile([128, 128], BF16, tag="jm")
        nc.sync.dma_start(out=jm_t[:], in_=jm_d)

        wqk_tiles = {}

        def load_wqk(p):
            t = wqkp.tile([128, KT, 256], BF16, tag="wqk", name=f"wqk{p}")
            nc.sync.dma_start(out=t[:], in_=wqk_r[:, :, 256 * p:256 * (p + 1)])
            wqk_tiles[p] = t

        xc_tiles = {}

        def load_xc(p, tq):
            t = xcp.tile([128, KT, XC], BF16, tag="xc", name=f"xc{p}_{tq}")
            nc.sync.dma_start(out=t[:], in_=xT_r[:, :, tq * XC:(tq + 1) * XC])
            xc_tiles[(p, tq)] = t

        load_wqk(0)
        load_xc(0, 0)
        cos_t = consts.tile([128, T], F32, tag="cos")
        nc.sync.dma_start(out=cos_t[:], in_=cos_d)
        sin_t = consts.tile([128, T], F32, tag="sin")
        nc.sync.dma_start(out=sin_t[:], in_=sin_d)
        wva_t = consts.tile([128, KT, 520], BF16, tag="wva")
        nc.sync.dma_start(out=wva_t[:], in_=wva_d.rearrange("(k p) m -> p k m", p=128))
        load_wqk(1)
        wo_t = consts.tile([128, 4, 1024], BF16, tag="wo")
        nc.sync.dma_start(out=wo_t[:], in_=wo_d.rearrange("(k p) m -> p k m", p=128))

        # additive causal masks: tri block [128,128] (valid iff c-r>=0) and
        # the d=3 variant [128,256] = [all-masked | tri]
        mask_t = consts.tile([128, 128], F32, tag="mask")
        nc.gpsimd.memset(mask_t[:], 0.0)
        nc.gpsimd.affine_select(
            out=mask_t[:], in_=mask_t[:], compare_op=ALU.is_ge, fill=MASK_VAL,
            base=0, pattern=[[1, 128]], channel_multiplier=-1)
        mask3_t = consts.tile([128, 256], F32, tag="mask3")
        nc.gpsimd.memset(mask3_t[:, 0:128], MASK_VAL)
        nc.gpsimd.memset(mask3_t[:, 128:256], 0.0)
        nc.gpsimd.affine_select(
            out=mask3_t[:, 128:256], in_=mask3_t[:, 128:256], compare_op=ALU.is_ge,
            fill=MASK_VAL, base=0, pattern=[[1, 128]], channel_multiplier=-1)

        # V_aug for all 16 t-blocks: [128 tok, 16 * (8 heads * 65)], bf16
        V_t = vpool.tile([128, NKB, 520], BF16, tag="V")

        # ---------------- pipeline unit generators ----------------
        qk_state = {}

        def qk_units(p):
            """8 units: qk-proj matmul chain + rope for (chunk, m)."""
            qp_ts = [qkp.tile([128, QC], BF16, tag="qp", name=f"qp{p}_{i}")
                     for i in range(NQC)]
            kp_ts = [qkp.tile([128, QC], BF16, tag="kp", name=f"kp{p}_{i}")
                     for i in range(NQC)]
            qk_state[p] = (qp_ts, kp_ts)
            for tq in range(NXC):
                for mloc in (0, 1):
                    def unit(tq=tq, mloc=mloc, qp_ts=qp_ts, kp_ts=kp_ts, p=p):
                        if mloc == 0 and tq + 1 < NXC:
                            load_xc(p, tq + 1)   # prefetch next chunk
                        c0 = tq * XC
                        dest = qp_ts if mloc == 0 else kp_ts
                        msel = 2 * p + mloc
                        mmp = ps_mm.tile([128, XC], F32, tag="mm")
                        wq = wqk_tiles[p]
                        xc = xc_tiles[(p, tq)]
                        for k in range(KT):
                            nc.tensor.matmul(
                                mmp[:], lhsT=wq[:, k, mloc * 128:(mloc + 1) * 128],
                                rhs=xc[:, k, :],
                                start=(k == 0), stop=(k == KT - 1))
                        bcol = bqk_t[:, msel:msel + 1]
                        # T1 = (psum + b) * cos ; T2 = (psum + b) * sin
                        t1 = t1p.tile([128, XC], BF16, tag="t1")
                        nc.vector.scalar_tensor_tensor(
                            t1[:], mmp[:], bcol, cos_t[:, c0:c0 + XC],
                            op0=ALU.add, op1=ALU.mult)
                        t2 = t1p.tile([128, XC], BF16, tag="t2")
                        nc.vector.scalar_tensor_tensor(
                            t2[:], mmp[:], bcol, sin_t[:, c0:c0 + XC],
                            op0=ALU.add, op1=ALU.mult)
                        # signed half-swap on the PE: t2s = J^T @ t2
                        t2s = ps_mm.tile([128, XC], F32, tag="mm", name="t2s")
                        nc.tensor.matmul(t2s[:], lhsT=jm_t[:], rhs=t2[:],
                                         start=True, stop=True)
                        nc.vector.tensor_add(dest[tq][:, 0:XC], t1[:], t2s[:])
                    yield unit

        def v_units():
            """32 units: v-proj half-chains per t-block (pair-0 chunks)."""
            for tb in range(NKB):
                for half in range(2):
                    def unit(tb=tb, half=half):
                        h0 = half * 260
                        xc = xc_tiles[(0, tb // 4)]
                        tb2 = tb % 4
                        pvm = ps_mm.tile([128, 260], F32, tag="mm", name="pvm")
                        for k in range(KT):
                            nc.tensor.matmul(
                                pvm[:], lhsT=xc[:, k, tb2 * 128:(tb2 + 1) * 128],
                                rhs=wva_t[:, k, h0:h0 + 260],
                                start=(k == 0), stop=False)
                        nc.tensor.matmul(pvm[:], lhsT=ones_t[:],
                                         rhs=bva_t[:, h0:h0 + 260],
                                         start=False, stop=True)
                        nc.scalar.copy(V_t[:, tb, h0:h0 + 260], pvm[:])
                    yield unit

        at_tiles = {}

        def out_proj_qc(qc):
            for qb2 in range(4):
                for oc in range(2):
                    po = ps_s.tile([128, 512], F32, tag="s", name="po")
                    for p4 in range(NPAIR):
                        nc.tensor.matmul(
                            po[:],
                            lhsT=at_tiles[p4][qc][:, qb2 * 128:qb2 * 128 + 128],
                            rhs=wo_t[:, p4, oc * 512:(oc + 1) * 512],
                            start=(p4 == 0), stop=(p4 == NPAIR - 1))
                    ot = outp.tile([128, 512], BF16, tag="ot")
                    nc.scalar.copy(ot[:], po[:])
                    qb = qc * 4 + qb2
                    nc.sync.dma_start(out=out_d[qb * 128:(qb + 1) * 128,
                                                oc * 512:(oc + 1) * 512], in_=ot[:])

        def attn_steps(p):
            """Yields ('first'|'blk'|'fin', qc, fn) steps for pair p."""
            qp_ts, kp_ts = qk_state[p]
            at_qs = [atp.tile([128, QC], BF16, tag="attnT", name=f"at{p}_{i}")
                     for i in range(NQC)]
            at_tiles[p] = at_qs
            for qc in range(NQC):
                nkb = 4 * qc + 4
                pvA = ps_pv.tile([65, QC], F32, tag="pv", name=f"pvA{p}_{qc}")
                pvB = ps_pv.tile([65, QC], F32, tag="pv", name=f"pvB{p}_{qc}")
                s_tiles = {}

                def emit_s(kb, qc=qc):
                    d = kb - 4 * qc
                    v0 = 0 if d < 0 else 128 * d
                    kq = kp_ts[kb // 4]
                    kc0 = (kb % 4) * 128
                    qq = qp_ts[qc]
                    tiles = []
                    for hh in range(2):
                        sh = ps_s.tile([128, QC], F32, tag="s", name=f"s{hh}")
                        nc.tensor.matmul(
                            sh[:, v0:], lhsT=kq[64 * hh:64 * hh + 64, kc0:kc0 + 128],
                            rhs=qq[64 * hh:64 * hh + 64, v0:],
                            start=True, stop=True, tile_position=(64 * hh, 0))
                        tiles.append(sh)
                    s_tiles[kb] = (tiles, d, v0)

                def first(qc=qc):
                    emit_s(0)
                    emit_s(1)
                yield ("first", qc, first)

                for kb in range(nkb):
                    def step(kb=kb, qc=qc, nkb=nkb, pvA=pvA, pvB=pvB, p=p):
                        tiles, d, v0 = s_tiles.pop(kb)
                        es = []
                        for hh in range(2):
                            sh = tiles[hh]
                            if d >= 0:
                                nc.vector.tensor_add(sh[:, v0:v0 + 128],
                                                     sh[:, v0:v0 + 128], mask_t[:])
                            e = ep.tile([128, QC], BF16, tag="e")
                            nc.scalar.activation(e[:, v0:], sh[:, v0:],
                                                 AF.Exp, scale=0.125)
                            es.append(e)
                        if kb + 2 < nkb:
                            emit_s(kb + 2)
                        for hh, pv in ((0, pvA), (1, pvB)):
                            nc.tensor.matmul(
                                pv[0:65, v0:],
                                lhsT=V_t[:, kb, (2 * p + hh) * 65:(2 * p + hh) * 65 + 65],
                                rhs=es[hh][:, v0:],
                                start=(kb == 0), stop=(kb == nkb - 1))
                    yield ("blk", qc, step)

                def finalize(qc=qc, pvA=pvA, pvB=pvB, at_qs=at_qs):
                    for hh, pv in ((0, pvA), (1, pvB)):
                        zrow = zbp.tile([1, QC], F32, tag="zrow",
                                        name=f"zrow{hh}")
                        nc.vector.tensor_copy(zrow[:], pv[64:65, :])
                        rz1 = zbp.tile([1, QC], F32, tag="rz1", name=f"rz1{hh}")
                        nc.vector.reciprocal_approx_fast(rz1[:], zrow[:])
                        rzb = zbp.tile([128, QC], F32, tag="rzb", name=f"rzb{hh}")
                        nc.gpsimd.partition_broadcast(rzb[:], rz1[:])
                        sl = at_qs[qc][64 * hh:64 * hh + 64, :]
                        if hh == 0:
                            nc.vector.tensor_mul(sl, pv[0:64, :], rzb[0:64, :])
                        else:
                            nc.vector.tensor_copy(sl, pv[0:64, :])
                            nc.vector.tensor_mul(sl, sl, rzb[64:128, :])
                yield ("fin", qc, finalize)

        # ---------------- pipeline schedule ----------------
        # stage 0: qk-proj(0) + v-proj interleaved (1 qk per 4 v units)
        qgen = qk_units(0)
        vgen = v_units()
        done = False
        while not done:
            u = next(qgen, None)
            if u is None:
                done = True
            else:
                u()
            for _ in range(4):
                u = next(vgen, None)
                if u:
                    u()
        for u in vgen:
            u()

        # stages 1..4: attention(p) with qk-proj(p+1) chains spliced in
        # every 5 key-blocks; stage 4 runs out-proj at chunk boundaries.
        for p in range(NPAIR):
            if p + 2 < NPAIR:
                load_wqk(p + 2)
            if p + 1 < NPAIR:
                fillers = list(qk_units(p + 1))
                load_xc(p + 1, 0)
            else:
                fillers = []
            nblk = 0
            for kind, qc, fn in attn_steps(p):
                if kind == "blk":
                    nblk += 1
                    if fillers and nblk % 5 == 0:
                        fillers.pop(0)()
                elif kind == "fin" and p == NPAIR - 1 and qc > 0:
                    out_proj_qc(qc - 1)
                fn()
            for f in fillers:
                f()
        out_proj_qc(NQC - 1)


def _prep_inputs(x, W_qkv, b_qkv, W_out, cos, sin):
    """Host-side sharding/permutation. Returns list of 8 per-core in_maps."""
    x = np.ascontiguousarray(np.asarray(x, dtype=np.float32))
    W_qkv = np.asarray(W_qkv, dtype=np.float32)
    b_qkv = np.asarray(b_qkv, dtype=np.float32)
    W_out = np.asarray(W_out, dtype=np.float32)
    cos = np.asarray(cos, dtype=np.float32)
    sin = np.asarray(sin, dtype=np.float32)

    import ml_dtypes
    BF = ml_dtypes.bfloat16
    xTs = [np.ascontiguousarray(x[b].T).astype(BF) for b in range(B)]
    # rope tables: rows r = table[:, r % 32]
    cosT = np.ascontiguousarray(cos.T)           # [32, T]
    sinT = np.ascontiguousarray(sin.T)
    cos4 = np.ascontiguousarray(np.tile(cosT, (4, 1)))   # [128, T]
    sin4 = np.ascontiguousarray(np.tile(sinT, (4, 1)))
    ones1 = np.ones((1, 128), BF)
    # signed half-swap for rope: t2s[r] = -t2[r+32] (r in lo half of each
    # 64-block), +t2[r-32] (hi half); t2s = J^T @ t2
    Jm = np.zeros((128, 128), np.float32)  # cast to BF below
    for blk in (0, 64):
        for i in range(32):
            Jm[blk + 32 + i, blk + i] = -1.0
            Jm[blk + i, blk + 32 + i] = 1.0

    groups = []
    for g in range(2):
        heads = [g * HPG + i for i in range(HPG)]
        qk_cols = []
        for p in range(NPAIR):
            A, Bh = heads[2 * p], heads[2 * p + 1]
            for base in (0, DK):                  # q block then k block
                for h in (A, Bh):
                    qk_cols += list(3 * DK * h + base + np.arange(0, DK, 2))
                    qk_cols += list(3 * DK * h + base + np.arange(1, DK, 2))
        qk_cols = np.array(qk_cols)
        wqk = np.ascontiguousarray(W_qkv[:, qk_cols]).astype(BF)  # [1024, 1024]
        bqk = np.ascontiguousarray(b_qkv[qk_cols].reshape(8, 128).T)  # [128, 8]
        # v with a normalizer ones col interleaved per head: [1024, 8*65]
        wva = np.zeros((D, 520), np.float32)
        bva = np.zeros((1, 520), np.float32)
        for i, h in enumerate(heads):
            vcols = 3 * DK * h + 2 * DK + np.arange(DK)
            wva[:, i * 65:i * 65 + 64] = W_qkv[:, vcols]
            bva[0, i * 65:i * 65 + 64] = b_qkv[vcols]
            bva[0, i * 65 + 64] = 1.0                 # ones column
        wo = np.ascontiguousarray(W_out[g * 512:(g + 1) * 512, :]).astype(BF)
        groups.append(dict(wqk=wqk, bqk=bqk, wva=wva.astype(BF),
                           bva=bva.astype(BF), wo=wo))

    in_maps = []
    for c in range(NC_):
        b, g = c // 2, c % 2
        gr = groups[g]
        in_maps.append({
            "xT": xTs[b], "wqk": gr["wqk"], "wva": gr["wva"], "bva": gr["bva"],
            "ones1": ones1, "wo": gr["wo"], "bqk": gr["bqk"],
            "cos4": cos4, "sin4": sin4, "Jmat": Jm.astype(BF),
        })
    return in_maps


def run(x, W_qkv, b_qkv, W_out, b_out, cos, sin, trace=False, trace_cores=None):
    """Build/compile (cached), run on 8 cores, return (out, BassKernelResults)."""
    if "nc" not in _cache:
        _cache["nc"] = _build_nc()
    nc = _cache["nc"]
    in_maps = _prep_inputs(x, W_qkv, b_qkv, W_out, cos, sin)
    kw = {}
    if trace:
        kw = dict(trace=True, trace_cores=trace_cores or [0])
    res = bass_utils.run_bass_kernel_spmd(nc, in_maps, core_ids=list(range(NC_)), **kw)
    b_out = np.asarray(b_out, dtype=np.float32)
    out = np.empty((B, T, D), np.float32)
    for b in range(B):
        out[b] = (res.results[2 * b]["out"].astype(np.float32)
                  + res.results[2 * b + 1]["out"].astype(np.float32)
                  + b_out[None, :])
    return out, res


def kernel(x, W_qkv, b_qkv, W_out, b_out, cos, sin):
    out, _ = run(x, W_qkv, b_qkv, W_out, b_out, cos, sin)
    return out


# revision 16
# speedup vs baseline: 1.3525x; 1.0986x over previous
"""Causal self-attention (B=4, T=2048, D=1024, H=16) on 8 TRN2 NeuronCores.

Sharding: core c handles batch b=c//2 and head-group g=c%2 (8 heads).
Each core computes its heads' attention + a partial output projection
(contraction over its 512 attn channels); the host sums the two partials
per batch and adds b_out.

v3: software-pipelined emission tuned for the TRN2 PE p-state (the
tensor engine only reaches 2.4 GHz after 3us of *continuous* busy; any
gap drops it to 1.2 GHz):
  stage 0    qk-proj(pair 0) + v-proj all 16 t-blocks, interleaved
  stage p    attention(pair p-1) with qk-proj(pair p) chains spliced in
             every ~5 key-blocks (covers the S->mask->exp->PV latency)
  stage 4    attention(pair 3) with out-proj of q-chunk qc-1 at each
             q-chunk boundary
Key moves vs the baseline:
  - rope's half-swap runs on the PE as a signed permutation matmul
    (J [128,128]); DVE rope is 2 STT + 1 full-width add per chunk
  - per-head S score tiles in a 4-deep PSUM ring -> 2-block lookahead
  - softmax normalizer 1/Z is computed off the PE-critical path; the
    pv PSUM ring is freed by cheap copies right at the chunk boundary
  - E and V are bf16 (cast on copy; same matmul rate, half the SBUF),
    output DMA is bf16
Pipeline rel-err ~3e-3 vs the 2e-2 gate.
"""
import sys
import numpy as np

for _p in ("/opt/trn_rl_repo", "/root/.axon_site/_ro/trn_rl_repo"):
    if _p not in sys.path:
        sys.path.append(_p)

import concourse.bass as bass
import concourse.bacc as bacc
import concourse.tile as tile
import concourse.mybir as mybir
from concourse import bass_utils

F32 = mybir.dt.float32
F32R = mybir.dt.float32r
BF16 = mybir.dt.bfloat16
AF = mybir.ActivationFunctionType
ALU = mybir.AluOpType

B, T, D, H, DK = 4, 2048, 1024, 16, 64
NC_ = 8          # cores
HPG = 8          # heads per group
NPAIR = 4        # head pairs per core
KT = 8           # 128-row k-tiles over D
XC = 512         # x/qkv t-chunk width
NXC = T // XC    # 4
QC = 512         # attention q-chunk width
NQC = T // QC    # 4
NKB = T // 128   # 16 key blocks
MASK_VAL = -30000.0

_cache = {}


def _build_nc(trace_scopes=False):
    nc = bacc.Bacc("TRN2", target_bir_lowering=False, debug=False)

    xT_d = nc.dram_tensor("xT", [D, T], BF16, kind="ExternalInput").ap()
    wqk_d = nc.dram_tensor("wqk", [D, 1024], BF16, kind="ExternalInput").ap()
    wva_d = nc.dram_tensor("wva", [D, 520], BF16, kind="ExternalInput").ap()
    bva_d = nc.dram_tensor("bva", [1, 520], BF16, kind="ExternalInput").ap()
    ones_d = nc.dram_tensor("ones1", [1, 128], BF16, kind="ExternalInput").ap()
    wo_d = nc.dram_tensor("wo", [512, 1024], BF16, kind="ExternalInput").ap()
    bqk_d = nc.dram_tensor("bqk", [128, 8], F32, kind="ExternalInput").ap()
    cos_d = nc.dram_tensor("cos4", [128, T], F32, kind="ExternalInput").ap()
    sin_d = nc.dram_tensor("sin4", [128, T], F32, kind="ExternalInput").ap()
    jm_d = nc.dram_tensor("Jmat", [128, 128], BF16, kind="ExternalInput").ap()
    out_d = nc.dram_tensor("out", [T, 1024], BF16, kind="ExternalOutput").ap()

    with tile.TileContext(nc, pool_alloc_mode="queue") as tc:
        _emit(tc, nc, xT_d, wqk_d, wva_d, bva_d, ones_d, wo_d, bqk_d,
              cos_d, sin_d, jm_d, out_d)
    nc.compile()
    return nc


def _emit(tc, nc, xT_d, wqk_d, wva_d, bva_d, ones_d, wo_d, bqk_d,
          cos_d, sin_d, jm_d, out_d):
    from contextlib import ExitStack
    ctx = ExitStack()
    with ctx:
        consts = ctx.enter_context(tc.tile_pool(name="consts", bufs=1))
        vpool = ctx.enter_context(tc.tile_pool(name="vpool", bufs=1))
        xcp = ctx.enter_context(tc.tile_pool(name="xcp", bufs=2))
        wqkp = ctx.enter_context(tc.tile_pool(name="wqkp", bufs=2))
        t1p = ctx.enter_context(tc.tile_pool(name="t1p", bufs=2))
        qkp = ctx.enter_context(tc.tile_pool(name="qkp", bufs=8))
        ep = ctx.enter_context(tc.tile_pool(name="ep", bufs=8))
        zbp = ctx.enter_context(tc.tile_pool(name="zbp", bufs=2))
        atp = ctx.enter_context(tc.tile_pool(name="atp", bufs=16))
        outp = ctx.enter_context(tc.tile_pool(name="outp", bufs=3))
        # PSUM: mm ring 2 (qk mmp / rope t2s / v pvm), s ring 4 (per-head
        # score tiles, boundary out-proj), pv ring 2 -> 2+4+2 = 8 banks
        ps_mm = ctx.enter_context(tc.tile_pool(name="ps_mm", bufs=2, space="PSUM"))
        ps_s = ctx.enter_context(tc.tile_pool(name="ps_s", bufs=4, space="PSUM"))
        ps_pv = ctx.enter_context(tc.tile_pool(name="ps_pv", bufs=2, space="PSUM"))

        wqk_r = wqk_d.rearrange("(k p) m -> p k m", p=128)
        xT_r = xT_d.rearrange("(k p) t -> p k t", p=128)

        # ---------------- constants / loads ----------------
        bqk_t = consts.tile([128, 8], F32, tag="bqk")
        nc.sync.dma_start(out=bqk_t[:], in_=bqk_d)
        ones_t = consts.tile([1, 128], BF16, tag="ones")
        nc.sync.dma_start(out=ones_t[:], in_=ones_d)
        bva_t = consts.tile([1, 520], BF16, tag="bva")
        nc.sync.dma_start(out=bva_t[:], in_=bva_d)
        jm_t = consts.tile([128, 128], BF16, tag="jm")
        nc.sync.dma_start(out=jm_t[:], in_=jm_d)

        wqk_tiles = {}

        def load_wqk(p):
            t = wqkp.tile([128, KT, 256], BF16, tag="wqk", name=f"wqk{p}")
            nc.sync.dma_start(out=t[:], in_=wqk_r[:, :, 256 * p:256 * (p + 1)])
            wqk_tiles[p] = t

        xc_tiles = {}

        def load_xc(p, tq):
            t = xcp.tile([128, KT, XC], BF16, tag="xc", name=f"xc{p}_{tq}")
            nc.sync.dma_start(out=t[:], in_=xT_r[:, :, tq * XC:(tq + 1) * XC])
            xc_tiles[(p, tq)] = t

        load_wqk(0)
        load_xc(0, 0)
        cos_t = consts.tile([128, T], F32, tag="cos")
        nc.sync.dma_start(out=cos_t[:], in_=cos_d)
        sin_t = consts.tile([128, T], F32, tag="sin")
        nc.sync.dma_start(out=sin_t[:], in_=sin_d)
        wva_t = consts.tile([128, KT, 520], BF16, tag="wva")
        nc.sync.dma_start(out=wva_t[:], in_=wva_d.rearrange("(k p) m -> p k m", p=128))
        load_wqk(1)
        wo_t = consts.tile([128, 4, 1024], BF16, tag="wo")
        nc.sync.dma_start(out=wo_t[:], in_=wo_d.rearrange("(k p) m -> p k m", p=128))

        # additive causal masks: tri block [128,128] (valid iff c-r>=0) and
        # the d=3 variant [128,256] = [all-masked | tri]
        mask_t = consts.tile([128, 128], F32, tag="mask")
        nc.gpsimd.memset(mask_t[:], 0.0)
        nc.gpsimd.affine_select(
            out=mask_t[:], in_=mask_t[:], compare_op=ALU.is_ge, fill=MASK_VAL,
            base=0, pattern=[[1, 128]], channel_multiplier=-1)
        mask3_t = consts.tile([128, 256], F32, tag="mask3")
        nc.gpsimd.memset(mask3_t[:, 0:128], MASK_VAL)
        nc.gpsimd.memset(mask3_t[:, 128:256], 0.0)
        nc.gpsimd.affine_select(
            out=mask3_t[:, 128:256], in_=mask3_t[:, 128:256], compare_op=ALU.is_ge,
            fill=MASK_VAL, base=0, pattern=[[1, 128]], channel_multiplier=-1)

        # V_aug for all 16 t-blocks: [128 tok, 16 * (8 heads * 65)], bf16
        V_t = vpool.tile([128, NKB, 520], BF16, tag="V")

        # ---------------- pipeline unit generators ----------------
        qk_state = {}

        def qk_units(p):
            """8 units: qk-proj matmul chain + rope for (chunk, m)."""
            qp_ts = [qkp.tile([128, QC], BF16, tag="qp", name=f"qp{p}_{i}")
                     for i in range(NQC)]
            kp_ts = [qkp.tile([128, QC], BF16, tag="kp", name=f"kp{p}_{i}")
                     for i in range(NQC)]
            qk_state[p] = (qp_ts, kp_ts)
            for tq in range(NXC):
                for mloc in (0, 1):
                    def unit(tq=tq, mloc=mloc, qp_ts=qp_ts, kp_ts=kp_ts, p=p):
                        if mloc == 0 and tq + 1 < NXC:
                            load_xc(p, tq + 1)   # prefetch next chunk
                        c0 = tq * XC
                        dest = qp_ts if mloc == 0 else kp_ts
                        msel = 2 * p + mloc
                        mmp = ps_mm.tile([128, XC], F32, tag="mm")
                        wq = wqk_tiles[p]
                        xc = xc_tiles[(p, tq)]
                        for k in range(KT):
                            nc.tensor.matmul(
                                mmp[:], lhsT=wq[:, k, mloc * 128:(mloc + 1) * 128],
                                rhs=xc[:, k, :],
                                start=(k == 0), stop=(k == KT - 1))
                        bcol = bqk_t[:, msel:msel + 1]
                        # T1 = (psum + b) * cos ; T2 = (psum + b) * sin
                        t1 = t1p.tile([128, XC], BF16, tag="t1")
                        nc.vector.scalar_tensor_tensor(
                            t1[:], mmp[:], bcol, cos_t[:, c0:c0 + XC],
                            op0=ALU.add, op1=ALU.mult)
                        t2 = t1p.tile([128, XC], BF16, tag="t2")
                        nc.vector.scalar_tensor_tensor(
                            t2[:], mmp[:], bcol, sin_t[:, c0:c0 + XC],
                            op0=ALU.add, op1=ALU.mult)
                        # signed half-swap on the PE: t2s = J^T @ t2
                        t2s = ps_mm.tile([128, XC], F32, tag="mm", name="t2s")
                        nc.tensor.matmul(t2s[:], lhsT=jm_t[:], rhs=t2[:],
                                         start=True, stop=True)
                        nc.vector.tensor_add(dest[tq][:, 0:XC], t1[:], t2s[:])
                    yield unit

        def v_units():
            """32 units: v-proj half-chains per t-block (pair-0 chunks)."""
            for tb in range(NKB):
                for half in range(2):
                    def unit(tb=tb, half=half):
                        h0 = half * 260
                        xc = xc_tiles[(0, tb // 4)]
                        tb2 = tb % 4
                        pvm = ps_mm.tile([128, 260], F32, tag="mm", name="pvm")
                        for k in range(KT):
                            nc.tensor.matmul(
                                pvm[:], lhsT=xc[:, k, tb2 * 128:(tb2 + 1) * 128],
                                rhs=wva_t[:, k, h0:h0 + 260],
                                start=(k == 0), stop=False)
                        nc.tensor.matmul(pvm[:], lhsT=ones_t[:],
                                         rhs=bva_t[:, h0:h0 + 260],
                                         start=False, stop=True)
                        nc.scalar.copy(V_t[:, tb, h0:h0 + 260], pvm[:])
                    yield unit

        at_tiles = {}

        def out_proj_qc(qc):
            for qb2 in range(4):
                for oc in range(2):
                    po = ps_s.tile([128, 512], F32, tag="s", name="po")
                    for p4 in range(NPAIR):
                        nc.tensor.matmul(
                            po[:],
                            lhsT=at_tiles[p4][qc][:, qb2 * 128:qb2 * 128 + 128],
                            rhs=wo_t[:, p4, oc * 512:(oc + 1) * 512],
                            start=(p4 == 0), stop=(p4 == NPAIR - 1))
                    ot = outp.tile([128, 512], BF16, tag="ot")
                    nc.scalar.copy(ot[:], po[:])
                    qb = qc * 4 + qb2
                    nc.sync.dma_start(out=out_d[qb * 128:(qb + 1) * 128,
                                                oc * 512:(oc + 1) * 512], in_=ot[:])

        def attn_steps(p):
            """Yields ('first'|'blk'|'fin', qc, fn) steps for pair p."""
            qp_ts, kp_ts = qk_state[p]
            at_qs = [atp.tile([128, QC], BF16, tag="attnT", name=f"at{p}_{i}")
                     for i in range(NQC)]
            at_tiles[p] = at_qs
            for qc in range(NQC):
                nkb = 4 * qc + 4
                pvA = ps_pv.tile([65, QC], F32, tag="pv", name=f"pvA{p}_{qc}")
                pvB = ps_pv.tile([65, QC], F32, tag="pv", name=f"pvB{p}_{qc}")
                s_tiles = {}

                def emit_s(kb, qc=qc):
                    d = kb - 4 * qc
                    v0 = 0 if d < 0 else 128 * d
                    kq = kp_ts[kb // 4]
                    kc0 = (kb % 4) * 128
                    qq = qp_ts[qc]
                    tiles = []
                    for hh in range(2):
                        sh = ps_s.tile([128, QC], F32, tag="s", name=f"s{hh}")
                        nc.tensor.matmul(
                            sh[:, v0:], lhsT=kq[64 * hh:64 * hh + 64, kc0:kc0 + 128],
                            rhs=qq[64 * hh:64 * hh + 64, v0:],
                            start=True, stop=True, tile_position=(64 * hh, 0))
                        tiles.append(sh)
                    s_tiles[kb] = (tiles, d, v0)

                def first(qc=qc):
                    emit_s(0)
                    emit_s(1)
                yield ("first", qc, first)

                for kb in range(nkb):
                    def step(kb=kb, qc=qc, nkb=nkb, pvA=pvA, pvB=pvB, p=p):
                        tiles, d, v0 = s_tiles.pop(kb)
                        es = []
                        for hh in range(2):
                            sh = tiles[hh]
                            if d >= 0:
                                nc.vector.tensor_add(sh[:, v0:v0 + 128],
                                                     sh[:, v0:v0 + 128], mask_t[:])
                            e = ep.tile([128, QC], BF16, tag="e")
                            nc.scalar.activation(e[:, v0:], sh[:, v0:],
                                                 AF.Exp, scale=0.125)
                            es.append(e)
                        if kb + 2 < nkb:
                            emit_s(kb + 2)
                        for hh, pv in ((0, pvA), (1, pvB)):
                            nc.tensor.matmul(
                                pv[0:65, v0:],
                                lhsT=V_t[:, kb, (2 * p + hh) * 65:(2 * p + hh) * 65 + 65],
                                rhs=es[hh][:, v0:],
                                start=(kb == 0), stop=(kb == nkb - 1))
                    yield ("blk", qc, step)

                def finalize(qc=qc, pvA=pvA, pvB=pvB, at_qs=at_qs):
                    for hh, pv in ((0, pvA), (1, pvB)):
                        zrow = zbp.tile([1, QC], F32, tag="zrow",
                                        name=f"zrow{hh}")
                        nc.vector.tensor_copy(zrow[:], pv[64:65, :])
                        rz1 = zbp.tile([1, QC], F32, tag="rz1", name=f"rz1{hh}")
                        nc.vector.reciprocal_approx_fast(rz1[:], zrow[:])
                        rzb = zbp.tile([128, QC], F32, tag="rzb", name=f"rzb{hh}")
                        nc.gpsimd.partition_broadcast(rzb[:], rz1[:])
                        sl = at_qs[qc][64 * hh:64 * hh + 64, :]
                        if hh == 0:
                            nc.vector.tensor_mul(sl, pv[0:64, :], rzb[0:64, :])
                        else:
                            nc.vector.tensor_copy(sl, pv[0:64, :])
                            nc.vector.tensor_mul(sl, sl, rzb[64:128, :])
                yield ("fin", qc, finalize)

        # ---------------- pipeline schedule ----------------
        # stage 0: qk-proj(0) + v-proj interleaved (1 qk per 4 v units)
        qgen = qk_units(0)
        vgen = v_units()
        done = False
        while not done:
            u = next(qgen, None)
            if u is None:
                done = True
            else:
                u()
            for _ in range(4):
                u = next(vgen, None)
                if u:
                    u()
        for u in vgen:
            u()

        # stages 1..4: attention(p) with qk-proj(p+1) chains spliced in
        # every 5 key-blocks; stage 4 runs out-proj at chunk boundaries.
        for p in range(NPAIR):
            if p + 2 < NPAIR:
                load_wqk(p + 2)
            if p + 1 < NPAIR:
                fillers = list(qk_units(p + 1))
                load_xc(p + 1, 0)
            else:
                fillers = []
            nblk = 0
            for kind, qc, fn in attn_steps(p):
                if kind == "blk":
                    nblk += 1
                    if fillers and nblk % 5 == 0:
                        fillers.pop(0)()
                elif kind == "fin" and p == NPAIR - 1 and qc > 0:
                    out_proj_qc(qc - 1)
                fn()
            for f in fillers:
                f()
        out_proj_qc(NQC - 1)


def _prep_inputs(x, W_qkv, b_qkv, W_out, cos, sin):
    """Host-side sharding/permutation. Returns list of 8 per-core in_maps."""
    x = np.ascontiguousarray(np.asarray(x, dtype=np.float32))
    W_qkv = np.asarray(W_qkv, dtype=np.float32)
    b_qkv = np.asarray(b_qkv, dtype=np.float32)
    W_out = np.asarray(W_out, dtype=np.float32)
    cos = np.asarray(cos, dtype=np.float32)
    sin = np.asarray(sin, dtype=np.float32)

    import ml_dtypes
    BF = ml_dtypes.bfloat16
    xTs = [np.ascontiguousarray(x[b].T).astype(BF) for b in range(B)]
    # rope tables: rows r = table[:, r % 32]
    cosT = np.ascontiguousarray(cos.T)           # [32, T]
    sinT = np.ascontiguousarray(sin.T)
    cos4 = np.ascontiguousarray(np.tile(cosT, (4, 1)))   # [128, T]
    sin4 = np.ascontiguousarray(np.tile(sinT, (4, 1)))
    ones1 = np.ones((1, 128), BF)
    # signed half-swap for rope: t2s[r] = -t2[r+32] (r in lo half of each
    # 64-block), +t2[r-32] (hi half); t2s = J^T @ t2
    Jm = np.zeros((128, 128), np.float32)  # cast to BF below
    for blk in (0, 64):
        for i in range(32):
            Jm[blk + 32 + i, blk + i] = -1.0
            Jm[blk + i, blk + 32 + i] = 1.0

    groups = []
    for g in range(2):
        heads = [g * HPG + i for i in range(HPG)]
        qk_cols = []
        for p in range(NPAIR):
            A, Bh = heads[2 * p], heads[2 * p + 1]
            for base in (0, DK):                  # q block then k block
                for h in (A, Bh):
                    qk_cols += list(3 * DK * h + base + np.arange(0, DK, 2))
                    qk_cols += list(3 * DK * h + base + np.arange(1, DK, 2))
        qk_cols = np.array(qk_cols)
        wqk = np.ascontiguousarray(W_qkv[:, qk_cols]).astype(BF)  # [1024, 1024]
        bqk = np.ascontiguousarray(b_qkv[qk_cols].reshape(8, 128).T)  # [128, 8]
        # v with a normalizer ones col interleaved per head: [1024, 8*65]
        wva = np.zeros((D, 520), np.float32)
        bva = np.zeros((1, 520), np.float32)
        for i, h in enumerate(heads):
            vcols = 3 * DK * h + 2 * DK + np.arange(DK)
            wva[:, i * 65:i * 65 + 64] = W_qkv[:, vcols]
            bva[0, i * 65:i * 65 + 64] = b_qkv[vcols]
            bva[0, i * 65 + 64] = 1.0                 # ones column
        wo = np.ascontiguousarray(W_out[g * 512:(g + 1) * 512, :]).astype(BF)
        groups.append(dict(wqk=wqk, bqk=bqk, wva=wva.astype(BF),
                           bva=bva.astype(BF), wo=wo))

    in_maps = []
    for c in range(NC_):
        b, g = c // 2, c % 2
        gr = groups[g]
        in_maps.append({
            "xT": xTs[b], "wqk": gr["wqk"], "wva": gr["wva"], "bva": gr["bva"],
            "ones1": ones1, "wo": gr["wo"], "bqk": gr["bqk"],
            "cos4": cos4, "sin4": sin4, "Jmat": Jm.astype(BF),
        })
    return in_maps


def run(x, W_qkv, b_qkv, W_out, b_out, cos, sin, trace=False, trace_cores=None):
    """Build/compile (cached), run on 8 cores, return (out, BassKernelResults)."""
    if "nc" not in _cache:
        _cache["nc"] = _build_nc()
    nc = _cache["nc"]
    in_maps = _prep_inputs(x, W_qkv, b_qkv, W_out, cos, sin)
    kw = {}
    if trace:
        kw = dict(trace=True, trace_cores=trace_cores or [0])
    res = bass_utils.run_bass_kernel_spmd(nc, in_maps, core_ids=list(range(NC_)), **kw)
    b_out = np.asarray(b_out, dtype=np.float32)
    out = np.empty((B, T, D), np.float32)
    for b in range(B):
        out[b] = (res.results[2 * b]["out"].astype(np.float32)
                  + res.results[2 * b + 1]["out"].astype(np.float32)
                  + b_out[None, :])
    return out, res


def kernel(x, W_qkv, b_qkv, W_out, b_out, cos, sin):
    out, _ = run(x, W_qkv, b_qkv, W_out, b_out, cos, sin)
    return out


# revision 18
# speedup vs baseline: 1.3562x; 1.0027x over previous
"""Causal self-attention (B=4, T=2048, D=1024, H=16) on 8 TRN2 NeuronCores.

Sharding: core c handles batch b=c//2 and head-group g=c%2 (8 heads).
Each core computes its heads' attention + a partial output projection
(contraction over its 512 attn channels); the host sums the two partials
per batch and adds b_out.

v3: software-pipelined emission tuned for the TRN2 PE p-state (the
tensor engine only reaches 2.4 GHz after 3us of *continuous* busy; any
gap drops it to 1.2 GHz):
  stage 0    qk-proj(pair 0) + v-proj all 16 t-blocks, interleaved
  stage p    attention(pair p-1) with qk-proj(pair p) chains spliced in
             every ~5 key-blocks (covers the S->mask->exp->PV latency)
  stage 4    attention(pair 3) with out-proj of q-chunk qc-1 at each
             q-chunk boundary
Key moves vs the baseline:
  - rope's half-swap runs on the PE as a signed permutation matmul
    (J [128,128]); DVE rope is 2 STT + 1 full-width add per chunk
  - per-head S score tiles in a 4-deep PSUM ring -> 2-block lookahead
  - softmax normalizer 1/Z is computed off the PE-critical path; the
    pv PSUM ring is freed by cheap copies right at the chunk boundary
  - E and V are bf16 (cast on copy; same matmul rate, half the SBUF),
    output DMA is bf16
Pipeline rel-err ~3e-3 vs the 2e-2 gate.
"""
import sys
import numpy as np

for _p in ("/opt/trn_rl_repo", "/root/.axon_site/_ro/trn_rl_repo"):
    if _p not in sys.path:
        sys.path.append(_p)

import concourse.bass as bass
import concourse.bacc as bacc
import concourse.tile as tile
import concourse.mybir as mybir
from concourse import bass_utils

F32 = mybir.dt.float32
F32R = mybir.dt.float32r
BF16 = mybir.dt.bfloat16
AF = mybir.ActivationFunctionType
ALU = mybir.AluOpType

B, T, D, H, DK = 4, 2048, 1024, 16, 64
NC_ = 8          # cores
HPG = 8          # heads per group
NPAIR = 4        # head pairs per core
KT = 8           # 128-row k-tiles over D
XC = 512         # x/qkv t-chunk width
NXC = T // XC    # 4
QC = 512         # attention q-chunk width
NQC = T // QC    # 4
NKB = T // 128   # 16 key blocks
MASK_VAL = -30000.0

_cache = {}


def _build_nc(trace_scopes=False):
    nc = bacc.Bacc("TRN2", target_bir_lowering=False, debug=False)

    xT_d = nc.dram_tensor("xT", [D, T], BF16, kind="ExternalInput").ap()
    wqk_d = nc.dram_tensor("wqk", [D, 1024], BF16, kind="ExternalInput").ap()
    wva_d = nc.dram_tensor("wva", [D, 520], BF16, kind="ExternalInput").ap()
    bva_d = nc.dram_tensor("bva", [1, 520], BF16, kind="ExternalInput").ap()
    ones_d = nc.dram_tensor("ones1", [1, 128], BF16, kind="ExternalInput").ap()
    wo_d = nc.dram_tensor("wo", [512, 1024], BF16, kind="ExternalInput").ap()
    bqk_d = nc.dram_tensor("bqk", [128, 8], F32, kind="ExternalInput").ap()
    cos_d = nc.dram_tensor("cos4", [128, T], F32, kind="ExternalInput").ap()
    sin_d = nc.dram_tensor("sin4", [128, T], F32, kind="ExternalInput").ap()
    jm_d = nc.dram_tensor("Jmat", [128, 128], BF16, kind="ExternalInput").ap()
    out_d = nc.dram_tensor("out", [T, 1024], BF16, kind="ExternalOutput").ap()

    with tile.TileContext(nc, pool_alloc_mode="queue") as tc:
        _emit(tc, nc, xT_d, wqk_d, wva_d, bva_d, ones_d, wo_d, bqk_d,
              cos_d, sin_d, jm_d, out_d)
    nc.compile()
    return nc


def _emit(tc, nc, xT_d, wqk_d, wva_d, bva_d, ones_d, wo_d, bqk_d,
          cos_d, sin_d, jm_d, out_d):
    from contextlib import ExitStack
    ctx = ExitStack()
    with ctx:
        consts = ctx.enter_context(tc.tile_pool(name="consts", bufs=1))
        vpool = ctx.enter_context(tc.tile_pool(name="vpool", bufs=1))
        xcp = ctx.enter_context(tc.tile_pool(name="xcp", bufs=3))
        wqkp = ctx.enter_context(tc.tile_pool(name="wqkp", bufs=2))
        t1p = ctx.enter_context(tc.tile_pool(name="t1p", bufs=4))
        qkp = ctx.enter_context(tc.tile_pool(name="qkp", bufs=8))
        ep = ctx.enter_context(tc.tile_pool(name="ep", bufs=8))
        zbp = ctx.enter_context(tc.tile_pool(name="zbp", bufs=2))
        atp = ctx.enter_context(tc.tile_pool(name="atp", bufs=16))
        outp = ctx.enter_context(tc.tile_pool(name="outp", bufs=4))
        # PSUM: mm ring 2 (qk mmp / rope t2s / v pvm), s ring 4 (per-head
        # score tiles, boundary out-proj), pv ring 2 -> 2+4+2 = 8 banks
        ps_mm = ctx.enter_context(tc.tile_pool(name="ps_mm", bufs=2, space="PSUM"))
        ps_s = ctx.enter_context(tc.tile_pool(name="ps_s", bufs=4, space="PSUM"))
        ps_pv = ctx.enter_context(tc.tile_pool(name="ps_pv", bufs=2, space="PSUM"))

        wqk_r = wqk_d.rearrange("(k p) m -> p k m", p=128)
        xT_r = xT_d.rearrange("(k p) t -> p k t", p=128)

        # ---------------- constants / loads ----------------
        bqk_t = consts.tile([128, 8], F32, tag="bqk")
        nc.sync.dma_start(out=bqk_t[:], in_=bqk_d)
        ones_t = consts.tile([1, 128], BF16, tag="ones")
        nc.sync.dma_start(out=ones_t[:], in_=ones_d)
        bva_t = consts.tile([1, 520], BF16, tag="bva")
        nc.sync.dma_start(out=bva_t[:], in_=bva_d)
        jm_t = consts.tile([128, 128], BF16, tag="jm")
        nc.sync.dma_start(out=jm_t[:], in_=jm_d)

        wqk_tiles = {}

        def load_wqk(p):
            t = wqkp.tile([128, KT, 256], BF16, tag="wqk", name=f"wqk{p}")
            nc.sync.dma_start(out=t[:], in_=wqk_r[:, :, 256 * p:256 * (p + 1)])
            wqk_tiles[p] = t

        xc_tiles = {}

        def load_xc(p, tq):
            t = xcp.tile([128, KT, XC], BF16, tag="xc", name=f"xc{p}_{tq}")
            nc.sync.dma_start(out=t[:], in_=xT_r[:, :, tq * XC:(tq + 1) * XC])
            xc_tiles[(p, tq)] = t

        load_wqk(0)
        load_xc(0, 0)
        cos_t = consts.tile([128, T], F32, tag="cos")
        nc.sync.dma_start(out=cos_t[:], in_=cos_d)
        sin_t = consts.tile([128, T], F32, tag="sin")
        nc.sync.dma_start(out=sin_t[:], in_=sin_d)
        wva_t = consts.tile([128, KT, 520], BF16, tag="wva")
        nc.sync.dma_start(out=wva_t[:], in_=wva_d.rearrange("(k p) m -> p k m", p=128))
        load_wqk(1)
        wo_t = consts.tile([128, 4, 1024], BF16, tag="wo")
        nc.sync.dma_start(out=wo_t[:], in_=wo_d.rearrange("(k p) m -> p k m", p=128))

        # additive causal masks: tri block [128,128] (valid iff c-r>=0) and
        # the d=3 variant [128,256] = [all-masked | tri]
        mask_t = consts.tile([128, 128], F32, tag="mask")
        nc.gpsimd.memset(mask_t[:], 0.0)
        nc.gpsimd.affine_select(
            out=mask_t[:], in_=mask_t[:], compare_op=ALU.is_ge, fill=MASK_VAL,
            base=0, pattern=[[1, 128]], channel_multiplier=-1)
        mask3_t = consts.tile([128, 256], F32, tag="mask3")
        nc.gpsimd.memset(mask3_t[:, 0:128], MASK_VAL)
        nc.gpsimd.memset(mask3_t[:, 128:256], 0.0)
        nc.gpsimd.affine_select(
            out=mask3_t[:, 128:256], in_=mask3_t[:, 128:256], compare_op=ALU.is_ge,
            fill=MASK_VAL, base=0, pattern=[[1, 128]], channel_multiplier=-1)

        # V_aug for all 16 t-blocks: [128 tok, 16 * (8 heads * 65)], bf16
        V_t = vpool.tile([128, NKB, 520], BF16, tag="V")

        # ---------------- pipeline unit generators ----------------
        qk_state = {}

        def qk_units(p):
            """8 units: qk-proj matmul chain + rope for (chunk, m)."""
            qp_ts = [qkp.tile([128, QC], BF16, tag="qp", name=f"qp{p}_{i}")
                     for i in range(NQC)]
            kp_ts = [qkp.tile([128, QC], BF16, tag="kp", name=f"kp{p}_{i}")
                     for i in range(NQC)]
            qk_state[p] = (qp_ts, kp_ts)
            for tq in range(NXC):
                for mloc in (0, 1):
                    def unit(tq=tq, mloc=mloc, qp_ts=qp_ts, kp_ts=kp_ts, p=p):
                        if mloc == 0 and tq + 1 < NXC:
                            load_xc(p, tq + 1)   # prefetch next chunk
                        c0 = tq * XC
                        dest = qp_ts if mloc == 0 else kp_ts
                        msel = 2 * p + mloc
                        mmp = ps_mm.tile([128, XC], F32, tag="mm")
                        wq = wqk_tiles[p]
                        xc = xc_tiles[(p, tq)]
                        for k in range(KT):
                            nc.tensor.matmul(
                                mmp[:], lhsT=wq[:, k, mloc * 128:(mloc + 1) * 128],
                                rhs=xc[:, k, :],
                                start=(k == 0), stop=(k == KT - 1))
                        bcol = bqk_t[:, msel:msel + 1]
                        # T1 = (psum + b) * cos ; T2 = (psum + b) * sin
                        t1 = t1p.tile([128, XC], BF16, tag="t1")
                        nc.vector.scalar_tensor_tensor(
                            t1[:], mmp[:], bcol, cos_t[:, c0:c0 + XC],
                            op0=ALU.add, op1=ALU.mult)
                        t2 = t1p.tile([128, XC], BF16, tag="t2")
                        nc.vector.scalar_tensor_tensor(
                            t2[:], mmp[:], bcol, sin_t[:, c0:c0 + XC],
                            op0=ALU.add, op1=ALU.mult)
                        # signed half-swap on the PE: t2s = J^T @ t2
                        t2s = ps_mm.tile([128, XC], F32, tag="mm", name="t2s")
                        nc.tensor.matmul(t2s[:], lhsT=jm_t[:], rhs=t2[:],
                                         start=True, stop=True)
                        nc.vector.tensor_add(dest[tq][:, 0:XC], t1[:], t2s[:])
                    yield unit

        def v_units():
            """32 units: v-proj half-chains per t-block (pair-0 chunks)."""
            for tb in range(NKB):
                for half in range(2):
                    def unit(tb=tb, half=half):
                        h0 = half * 260
                        xc = xc_tiles[(0, tb // 4)]
                        tb2 = tb % 4
                        pvm = ps_mm.tile([128, 260], F32, tag="mm", name="pvm")
                        for k in range(KT):
                            nc.tensor.matmul(
                                pvm[:], lhsT=xc[:, k, tb2 * 128:(tb2 + 1) * 128],
                                rhs=wva_t[:, k, h0:h0 + 260],
                                start=(k == 0), stop=False)
                        nc.tensor.matmul(pvm[:], lhsT=ones_t[:],
                                         rhs=bva_t[:, h0:h0 + 260],
                                         start=False, stop=True)
                        nc.scalar.copy(V_t[:, tb, h0:h0 + 260], pvm[:])
                    yield unit

        at_tiles = {}

        def out_proj_qc(qc):
            for qb2 in range(4):
                for oc in range(2):
                    po = ps_s.tile([128, 512], F32, tag="s", name="po")
                    for p4 in range(NPAIR):
                        nc.tensor.matmul(
                            po[:],
                            lhsT=at_tiles[p4][qc][:, qb2 * 128:qb2 * 128 + 128],
                            rhs=wo_t[:, p4, oc * 512:(oc + 1) * 512],
                            start=(p4 == 0), stop=(p4 == NPAIR - 1))
                    ot = outp.tile([128, 512], BF16, tag="ot")
                    nc.scalar.copy(ot[:], po[:])
                    qb = qc * 4 + qb2
                    nc.sync.dma_start(out=out_d[qb * 128:(qb + 1) * 128,
                                                oc * 512:(oc + 1) * 512], in_=ot[:])

        def attn_steps(p):
            """Yields ('first'|'blk'|'fin', qc, fn) steps for pair p."""
            qp_ts, kp_ts = qk_state[p]
            at_qs = [atp.tile([128, QC], BF16, tag="attnT", name=f"at{p}_{i}")
                     for i in range(NQC)]
            at_tiles[p] = at_qs
            for qc in range(NQC):
                nkb = 4 * qc + 4
                pvA = ps_pv.tile([65, QC], F32, tag="pv", name=f"pvA{p}_{qc}")
                pvB = ps_pv.tile([65, QC], F32, tag="pv", name=f"pvB{p}_{qc}")
                s_tiles = {}

                def emit_s(kb, qc=qc):
                    d = kb - 4 * qc
                    v0 = 0 if d < 0 else 128 * d
                    kq = kp_ts[kb // 4]
                    kc0 = (kb % 4) * 128
                    qq = qp_ts[qc]
                    tiles = []
                    for hh in range(2):
                        sh = ps_s.tile([128, QC], F32, tag="s", name=f"s{hh}")
                        nc.tensor.matmul(
                            sh[:, v0:], lhsT=kq[64 * hh:64 * hh + 64, kc0:kc0 + 128],
                            rhs=qq[64 * hh:64 * hh + 64, v0:],
                            start=True, stop=True, tile_position=(64 * hh, 0))
                        tiles.append(sh)
                    s_tiles[kb] = (tiles, d, v0)

                def first(qc=qc):
                    emit_s(0)
                    emit_s(1)
                yield ("first", qc, first)

                for kb in range(nkb):
                    def step(kb=kb, qc=qc, nkb=nkb, pvA=pvA, pvB=pvB, p=p):
                        tiles, d, v0 = s_tiles.pop(kb)
                        es = []
                        for hh in range(2):
                            sh = tiles[hh]
                            if d >= 0:
                                nc.vector.tensor_add(sh[:, v0:v0 + 128],
                                                     sh[:, v0:v0 + 128], mask_t[:])
                            e = ep.tile([128, QC], BF16, tag="e")
                            nc.scalar.activation(e[:, v0:], sh[:, v0:],
                                                 AF.Exp, scale=0.125)
                            es.append(e)
                        if kb + 2 < nkb:
                            emit_s(kb + 2)
                        for hh, pv in ((0, pvA), (1, pvB)):
                            nc.tensor.matmul(
                                pv[0:65, v0:],
                                lhsT=V_t[:, kb, (2 * p + hh) * 65:(2 * p + hh) * 65 + 65],
                                rhs=es[hh][:, v0:],
                                start=(kb == 0), stop=(kb == nkb - 1))
                    yield ("blk", qc, step)

                def finalize(qc=qc, pvA=pvA, pvB=pvB, at_qs=at_qs):
                    for hh, pv in ((0, pvA), (1, pvB)):
                        zrow = zbp.tile([1, QC], F32, tag="zrow",
                                        name=f"zrow{hh}")
                        nc.vector.tensor_copy(zrow[:], pv[64:65, :])
                        rz1 = zbp.tile([1, QC], F32, tag="rz1", name=f"rz1{hh}")
                        nc.vector.reciprocal_approx_fast(rz1[:], zrow[:])
                        rzb = zbp.tile([128, QC], F32, tag="rzb", name=f"rzb{hh}")
                        nc.gpsimd.partition_broadcast(rzb[:], rz1[:])
                        sl = at_qs[qc][64 * hh:64 * hh + 64, :]
                        if hh == 0:
                            nc.vector.tensor_mul(sl, pv[0:64, :], rzb[0:64, :])
                        else:
                            nc.vector.tensor_copy(sl, pv[0:64, :])
                            nc.vector.tensor_mul(sl, sl, rzb[64:128, :])
                yield ("fin", qc, finalize)

        # ---------------- pipeline schedule ----------------
        # stage 0: qk-proj(0) + v-proj interleaved (1 qk per 4 v units)
        qgen = qk_units(0)
        vgen = v_units()
        done = False
        while not done:
            u = next(qgen, None)
            if u is None:
                done = True
            else:
                u()
            for _ in range(4):
                u = next(vgen, None)
                if u:
                    u()
        for u in vgen:
            u()

        # stages 1..4: attention(p) with qk-proj(p+1) chains spliced in
        # every 5 key-blocks; stage 4 runs out-proj at chunk boundaries.
        for p in range(NPAIR):
            if p + 2 < NPAIR:
                load_wqk(p + 2)
            if p + 1 < NPAIR:
                fillers = list(qk_units(p + 1))
                load_xc(p + 1, 0)
            else:
                fillers = []
            nblk = 0
            for kind, qc, fn in attn_steps(p):
                if kind == "blk":
                    nblk += 1
                    if fillers and nblk % 5 == 0:
                        fillers.pop(0)()
                elif kind == "fin" and p == NPAIR - 1 and qc > 0:
                    out_proj_qc(qc - 1)
                fn()
            for f in fillers:
                f()
        out_proj_qc(NQC - 1)


def _prep_inputs(x, W_qkv, b_qkv, W_out, cos, sin):
    """Host-side sharding/permutation. Returns list of 8 per-core in_maps."""
    x = np.ascontiguousarray(np.asarray(x, dtype=np.float32))
    W_qkv = np.asarray(W_qkv, dtype=np.float32)
    b_qkv = np.asarray(b_qkv, dtype=np.float32)
    W_out = np.asarray(W_out, dtype=np.float32)
    cos = np.asarray(cos, dtype=np.float32)
    sin = np.asarray(sin, dtype=np.float32)

    import ml_dtypes
    BF = ml_dtypes.bfloat16
    xTs = [np.ascontiguousarray(x[b].T).astype(BF) for b in range(B)]
    # rope tables: rows r = table[:, r % 32]
    cosT = np.ascontiguousarray(cos.T)           # [32, T]
    sinT = np.ascontiguousarray(sin.T)
    cos4 = np.ascontiguousarray(np.tile(cosT, (4, 1)))   # [128, T]
    sin4 = np.ascontiguousarray(np.tile(sinT, (4, 1)))
    ones1 = np.ones((1, 128), BF)
    # signed half-swap for rope: t2s[r] = -t2[r+32] (r in lo half of each
    # 64-block), +t2[r-32] (hi half); t2s = J^T @ t2
    Jm = np.zeros((128, 128), np.float32)  # cast to BF below
    for blk in (0, 64):
        for i in range(32):
            Jm[blk + 32 + i, blk + i] = -1.0
            Jm[blk + i, blk + 32 + i] = 1.0

    groups = []
    for g in range(2):
        heads = [g * HPG + i for i in range(HPG)]
        qk_cols = []
        for p in range(NPAIR):
            A, Bh = heads[2 * p], heads[2 * p + 1]
            for base in (0, DK):                  # q block then k block
                for h in (A, Bh):
                    qk_cols += list(3 * DK * h + base + np.arange(0, DK, 2))
                    qk_cols += list(3 * DK * h + base + np.arange(1, DK, 2))
        qk_cols = np.array(qk_cols)
        wqk = np.ascontiguousarray(W_qkv[:, qk_cols]).astype(BF)  # [1024, 1024]
        bqk = np.ascontiguousarray(b_qkv[qk_cols].reshape(8, 128).T)  # [128, 8]
        # v with a normalizer ones col interleaved per head: [1024, 8*65]
        wva = np.zeros((D, 520), np.float32)
        bva = np.zeros((1, 520), np.float32)
        for i, h in enumerate(heads):
            vcols = 3 * DK * h + 2 * DK + np.arange(DK)
            wva[:, i * 65:i * 65 + 64] = W_qkv[:, vcols]
            bva[0, i * 65:i * 65 + 64] = b_qkv[vcols]
            bva[0, i * 65 + 64] = 1.0                 # ones column
        wo = np.ascontiguousarray(W_out[g * 512:(g + 1) * 512, :]).astype(BF)
        groups.append(dict(wqk=wqk, bqk=bqk, wva=wva.astype(BF),
                           bva=bva.astype(BF), wo=wo))

    in_maps = []
    for c in range(NC_):
        b, g = c // 2, c % 2
        gr = groups[g]
        in_maps.append({
            "xT": xTs[b], "wqk": gr["wqk"], "wva": gr["wva"], "bva": gr["bva"],
            "ones1": ones1, "wo": gr["wo"], "bqk": gr["bqk"],
            "cos4": cos4, "sin4": sin4, "Jmat": Jm.astype(BF),
        })
    return in_maps


def run(x, W_qkv, b_qkv, W_out, b_out, cos, sin, trace=False, trace_cores=None):
    """Build/compile (cached), run on 8 cores, return (out, BassKernelResults)."""
    if "nc" not in _cache:
        _cache["nc"] = _build_nc()
    nc = _cache["nc"]
    in_maps = _prep_inputs(x, W_qkv, b_qkv, W_out, cos, sin)
    kw = {}
    if trace:
        kw = dict(trace=True, trace_cores=trace_cores or [0])
    res = bass_utils.run_bass_kernel_spmd(nc, in_maps, core_ids=list(range(NC_)), **kw)
    b_out = np.asarray(b_out, dtype=np.float32)
    out = np.empty((B, T, D), np.float32)
    for b in range(B):
        out[b] = (res.results[2 * b]["out"].astype(np.float32)
                  + res.results[2 * b + 1]["out"].astype(np.float32)
                  + b_out[None, :])
    return out, res


def kernel(x, W_qkv, b_qkv, W_out, b_out, cos, sin):
    out, _ = run(x, W_qkv, b_qkv, W_out, b_out, cos, sin)
    return out
